# revision 10
# baseline (speedup 1.0000x reference)
"""Trainium2 Bass kernel for nn_MeshGraphBlock (GNN message-passing block).

Computes, for x:[B,N,D], edges (src,dst):[E], degree:[N]:
    neighbor = scatter_add(x[:, src, :] -> dst) / clip(degree, 1)
    h  = concat(LN(x; sn_g, sn_b), LN(neighbor; nn_g, nn_b))   # [B,N,2D]
    h  = gelu_erf(h @ W1 + b1)                                  # [B,N,2D]
    y  = x + h @ W2 + b2                                        # [B,N,D]

Strategy (8 NeuronCores, SPMD, one compiled program; only data differs):
 - Destination-node tiles (128 nodes) dealt to cores by sorted round-robin.
 - Host pre-sorts edges by dst, packs x as a bf16 [N,2D] gather table (two
   int16-indexed halves), emits per-core gather indices with -1 padding
   (padded descriptors are skipped by the SWDGE ucode, so gather DMA pays
   for real edges only).
 - On device: dma_gather edge messages (512B rows); one-hot scatter
   matrices generated with a single broadcast-compare DVE op per position;
   scatter-add via PE matmuls into PSUM; LayerNorm mean/rstd from
   bn_stats on PSUM + a bit-hack Newton rsqrt on DVE; LN applied for free
   inside the Act-engine PSUM evacuation (per-partition scale/bias).
 - MLP runs fully in bf16 (PE transposes, both matmuls, residual read),
   activation table never swaps (gelu/identity/copy only).
"""

import math

import numpy as np
import ml_dtypes

P = 128
NCORES = 8
SPLIT = 32768           # int16 gather-index limit
CHUNK = 1024            # max idxs per dma_gather call (SWDGE ring)
SCRATCH = 16384         # dynamic_dma_scratch_size -> 1024-descriptor ring
GBUFS = 5               # gather-buffer pool depth (first GBUFS positions
                        # gather their padding too, to init SBUF)
SBUFS = 3
RSG_G = 6               # positions per rstd group

_CACHE = {}


def _prep(x, edge_src, edge_dst, degree):
    """Host-side sharding. Returns (structure, per-core inputs, tids)."""
    Bb, N, D = x.shape
    es = np.asarray(edge_src).astype(np.int64).ravel()
    ed = np.asarray(edge_dst).astype(np.int64).ravel()
    deg = np.asarray(degree).astype(np.float32).ravel()

    ntiles = math.ceil(N / P)
    ntiles_pad = math.ceil(ntiles / NCORES) * NCORES
    NTC = ntiles_pad // NCORES

    order = np.argsort(ed, kind="stable")
    ed_s = ed[order]
    es_s = es[order]
    bounds = np.searchsorted(ed_s, np.arange(ntiles_pad + 1) * P)

    counts = bounds[1:] - bounds[:-1]
    ranked = np.argsort(-counts, kind="stable")
    tids = [[0] * NTC for _ in range(NCORES)]
    for i, t in enumerate(ranked):
        tids[i % NCORES][i // NCORES] = int(t)

    # per (core, pos): G0 (src < SPLIT) and G1 index/dst-local streams
    gi = {}
    dli = {}
    for c in range(NCORES):
        for k in range(NTC):
            t = tids[c][k]
            a, b = bounds[t], bounds[t + 1]
            srcs = es_s[a:b]
            dloc = (ed_s[a:b] - t * P).astype(np.float32)
            m0 = srcs < SPLIT
            gi[c, k, 0] = srcs[m0].astype(np.int16)
            gi[c, k, 1] = (srcs[~m0] - SPLIT).astype(np.int16)
            dli[c, k, 0] = dloc[m0]
            dli[c, k, 1] = dloc[~m0]

    nmax = {(k, g): max(len(gi[c, k, g]) for c in range(NCORES))
            for k in range(NTC) for g in (0, 1)}
    T0 = [math.ceil(nmax[k, 0] / P) for k in range(NTC)]
    T1 = [math.ceil(nmax[k, 1] / P) for k in range(NTC)]
    TS = [t0 + t1 for t0, t1 in zip(T0, T1)]
    dl_off = np.concatenate([[0], np.cumsum(TS)])
    TTOT = int(dl_off[-1])

    idx_flat = np.zeros((NCORES, TTOT * P), dtype=np.int16)
    dl_flat = np.full((NCORES, TTOT * P), -1.0, dtype=np.float32)
    calls = []  # (pos, grp, slot_off, nt, idx_off, num) uniform across cores
    for k in range(NTC):
        for g, nt in ((0, T0[k]), (1, T1[k])):
            if nt == 0:
                continue
            so = int(dl_off[k]) + (T0[k] if g else 0)
            o = so * P
            for c in range(NCORES):
                ii = gi[c, k, g]
                dd = dli[c, k, g]
                idx_flat[c, o : o + len(ii)] = ii
                dl_flat[c, o : o + len(dd)] = dd
            num = math.ceil(nmax[k, g] / 16) * 16  # ucode reads 16-groups
            if k < GBUFS:
                # prime the gather pool: gather the padding too (idx 0)
                for c in range(NCORES):
                    idx_flat[c, o + len(gi[c, k, g]) : o + nt * P] = 0
                num = nt * P
            done = 0
            while done < num:
                n = min(num - done, CHUNK)
                calls.append((k, g, so + done // P, math.ceil(n / P),
                              o + done, n))
                done += n

    idx_wrapped = np.stack(
        [np.tile(idx_flat[c].reshape(-1, 16).T, (8, 1)) for c in range(NCORES)]
    )
    dlb = np.stack(
        [np.ascontiguousarray(dl_flat[c].reshape(TTOT, P).T).astype(
            ml_dtypes.bfloat16) for c in range(NCORES)]
    )  # [NCORES, 128, TTOT] bf16

    deg_r = np.ones((NCORES, P, NTC), dtype=np.float32)
    xs = np.zeros((NCORES, NTC * P, 2 * D), dtype=ml_dtypes.bfloat16)
    xf = np.asarray(x, dtype=np.float32)
    xcat = np.concatenate([xf[0], xf[1]], axis=1).astype(ml_dtypes.bfloat16)
    for c in range(NCORES):
        for k in range(NTC):
            t = tids[c][k]
            n0 = t * P
            n1 = min(n0 + P, N)
            if n1 <= n0:
                continue
            deg_r[c, : n1 - n0, k] = deg[n0:n1]
            xs[c, k * P : k * P + (n1 - n0), :] = xcat[n0:n1]

    # gather tables (rows %128-padded so whole-tile loads stay in range)
    xpa = np.ascontiguousarray(xcat[:SPLIT])
    nb_rows = math.ceil((ntiles_pad * P - SPLIT) / P) * P
    xpb = np.zeros((nb_rows, 2 * D), dtype=ml_dtypes.bfloat16)
    xpb[: N - SPLIT] = xcat[SPLIT:]

    struct = dict(NTC=NTC, T0=tuple(T0), T1=tuple(T1), TS=tuple(TS),
                  TTOT=TTOT, calls=tuple(calls),
                  dl_off=tuple(int(v) for v in dl_off),
                  NA=xpa.shape[0], NB=xpb.shape[0], D=D, Bb=Bb)
    percore = dict(idx=idx_wrapped, dlb=dlb, deg=deg_r, xs=xs)
    shared = dict(xpa=xpa, xpb=xpb)
    return struct, percore, shared, tids, N


def _build(struct):
    import concourse.bacc as bacc
    import concourse.tile as tile
    from concourse import mybir
    from concourse.masks import make_identity

    NTC, T0, T1, TS = struct["NTC"], struct["T0"], struct["T1"], struct["TS"]
    TTOT = struct["TTOT"]
    dl_off = struct["dl_off"]
    calls = struct["calls"]
    D = struct["D"]
    D2 = 2 * D
    TSMAX = max(TS)
    f32, bf16, i16 = mybir.dt.float32, mybir.dt.bfloat16, mybir.dt.int16
    i32 = mybir.dt.int32
    AOP = mybir.AluOpType
    AF = mybir.ActivationFunctionType

    calls_by_pos = {}
    for (k, g, so, nt, io, num) in calls:
        calls_by_pos.setdefault(k, []).append((g, so, nt, io, num))

    nc = bacc.Bacc("TRN2", target_bir_lowering=False, debug=False,
                   dynamic_dma_scratch_size=SCRATCH)
    d_xpa = nc.dram_tensor("xpa", [struct["NA"], D2], bf16, kind="ExternalInput")
    d_xpb = nc.dram_tensor("xpb", [struct["NB"], D2], bf16, kind="ExternalInput")
    d_xs = nc.dram_tensor("xs", [NTC * P, D2], bf16, kind="ExternalInput")
    d_idx = nc.dram_tensor("idx", [P, TTOT * 8], i16, kind="ExternalInput")
    d_dlb = nc.dram_tensor("dlb", [P, TTOT], bf16, kind="ExternalInput")
    d_deg = nc.dram_tensor("deg", [P, NTC], f32, kind="ExternalInput")
    d_w1 = nc.dram_tensor("w1", [D2, D2], f32, kind="ExternalInput")
    d_w2 = nc.dram_tensor("w2b", [D2, D], bf16, kind="ExternalInput")
    d_b1 = nc.dram_tensor("b1r", [P, 2], f32, kind="ExternalInput")
    d_b2 = nc.dram_tensor("b2r", [1, P], bf16, kind="ExternalInput")
    d_gx = nc.dram_tensor("gx", [P, 1], f32, kind="ExternalInput")
    d_gn = nc.dram_tensor("gn", [P, 1], f32, kind="ExternalInput")
    d_bx = nc.dram_tensor("bx", [P, 1], f32, kind="ExternalInput")
    d_bn = nc.dram_tensor("bn", [P, 1], f32, kind="ExternalInput")
    d_y = nc.dram_tensor("y", [NTC * P, D2], bf16, kind="ExternalOutput")

    with tile.TileContext(nc) as tc:
        with (
            tc.tile_pool(name="const", bufs=1) as cp,
            tc.tile_pool(name="gath", bufs=GBUFS) as gpool,
            tc.tile_pool(name="sel", bufs=SBUFS) as spool,
            tc.tile_pool(name="work", bufs=3) as wp,
            tc.tile_pool(name="grp", bufs=2) as gw,
            tc.tile_pool(name="nbps", bufs=2, space="PSUM") as nbps,
            tc.tile_pool(name="trps", bufs=2, space="PSUM") as trps,
            tc.tile_pool(name="mm1ps", bufs=2, space="PSUM") as mm1ps,
            tc.tile_pool(name="mm2ps", bufs=2, space="PSUM") as mm2ps,
        ):
            # ---- one-time constants ----
            idx_sb = cp.tile([P, TTOT * 8], i16)
            nc.sync.dma_start(idx_sb[:], d_idx.ap())
            dlb_sb = cp.tile([P, TTOT], bf16)
            nc.sync.dma_start(dlb_sb[:], d_dlb.ap())
            deg_sb = cp.tile([P, NTC], f32)
            nc.sync.dma_start(deg_sb[:], d_deg.ap())
            invd = cp.tile([P, NTC], f32)
            nc.vector.tensor_scalar_max(invd[:], deg_sb[:], 1.0)
            nc.vector.reciprocal(invd[:], invd[:])
            invd2 = cp.tile([P, NTC], f32)
            nc.vector.tensor_tensor(invd2[:], invd[:], invd[:], op=AOP.mult)

            identb = cp.tile([P, P], bf16)
            make_identity(nc, identb[:])
            iota3 = cp.tile([P, TSMAX, P], bf16)
            nc.gpsimd.iota(iota3[:], pattern=[[0, TSMAX], [1, P]], base=0,
                           channel_multiplier=0,
                           allow_small_or_imprecise_dtypes=True)

            gx_sb = cp.tile([P, 1], f32); nc.sync.dma_start(gx_sb[:], d_gx.ap())
            gn_sb = cp.tile([P, 1], f32); nc.sync.dma_start(gn_sb[:], d_gn.ap())
            bx_sb = cp.tile([P, 1], f32); nc.sync.dma_start(bx_sb[:], d_bx.ap())
            bn_sb = cp.tile([P, 1], f32); nc.sync.dma_start(bn_sb[:], d_bn.ap())
            b1r_sb = cp.tile([P, 2], f32); nc.sync.dma_start(b1r_sb[:], d_b1.ap())
            b2r_sb = cp.tile([1, P], bf16); nc.sync.dma_start(b2r_sb[:], d_b2.ap())
            ones1 = cp.tile([1, P], bf16)
            nc.vector.memset(ones1[:], 1.0)

            # W1 f32 tiles (for b1 fold), gamma-scaled bf16 copies, W2 bf16
            w1t = [[cp.tile([P, P], f32, name=f"w1t{kt}{jt}") for jt in range(2)]
                   for kt in range(2)]
            w1s = [[cp.tile([P, P], bf16, name=f"w1s{kt}{jt}") for jt in range(2)]
                   for kt in range(2)]
            gam = [gx_sb, gn_sb]
            for kt in range(2):
                for jt in range(2):
                    nc.sync.dma_start(
                        w1t[kt][jt][:],
                        d_w1[kt * P : (kt + 1) * P, jt * P : (jt + 1) * P],
                    )
                    nc.vector.tensor_scalar_mul(
                        w1s[kt][jt][:], w1t[kt][jt][:], gam[kt][:]
                    )
            w2t = [cp.tile([P, P], bf16, name=f"w2t{kt}") for kt in range(2)]
            for kt in range(2):
                nc.sync.dma_start(w2t[kt][:], d_w2[kt * P : (kt + 1) * P, :])

            # b1_eff = b1 + beta_cat @ W1  (per-partition layout [128, jt])
            bet = [bx_sb, bn_sb]
            b1b_ps = mm1ps.tile([P, 2], f32, space="PSUM", tag="m1")
            for jt in range(2):
                for kt in range(2):
                    nc.tensor.matmul(
                        b1b_ps[:, jt : jt + 1], lhsT=w1t[kt][jt][:],
                        rhs=bet[kt][:], start=(kt == 0), stop=(kt == 1),
                    )
            b1e_sb = cp.tile([P, 2], f32)
            nc.vector.tensor_add(b1e_sb[:], b1b_ps[:], b1r_sb[:])

            # resident x slices: [128, NTC, 2D] bf16
            xs_sb = cp.tile([P, NTC, D2], bf16)
            for k in range(NTC):
                nc.sync.dma_start(xs_sb[:, k, :], d_xs[k * P : (k + 1) * P, :])

            # ---- main loop: groups of RSG_G positions ----
            for k0 in range(0, NTC, RSG_G):
                gs = min(RSG_G, NTC - k0)
                nb_t = {}
                # stats layout: x entries [2k], nb entries [2G + 2k]
                mvg = gw.tile([P, 4 * gs, 2], f32, tag="mvg", name=f"mvg{k0}")

                # phase A: gather, scatter-add, stats
                for gi_ in range(gs):
                    k = k0 + gi_
                    slots = TS[k]
                    g = gpool.tile([P, TSMAX, D2], bf16, tag="g", name=f"g{k}")
                    for (grp, so, nt, io, num) in calls_by_pos.get(k, []):
                        src_t = d_xpa if grp == 0 else d_xpb
                        so_l = so - dl_off[k]
                        nc.gpsimd.dma_gather(
                            g[:, so_l : so_l + nt, :], src_t.ap(),
                            idx_sb[:, io // 16 : (io + nt * P) // 16],
                            num, num, D2, single_packet=False,
                        )
                    S = spool.tile([P, TSMAX, P], bf16, tag="S", name=f"S{k}")
                    dlb_b = dlb_sb[:, dl_off[k] : dl_off[k] + slots, None] \
                        .broadcast_to((P, slots, P))
                    nc.vector.scalar_tensor_tensor(
                        out=S[:, :slots, :], in0=iota3[:, :slots, :],
                        scalar=0.0, in1=dlb_b,
                        op0=AOP.bypass, op1=AOP.is_equal)
                    nb_ps = nbps.tile([P, D2], f32, space="PSUM", tag="nbp",
                                      name=f"nbp{k}")
                    for t in range(slots):
                        nc.tensor.matmul(
                            nb_ps[:], lhsT=S[:, t, :], rhs=g[:, t, :],
                            start=(t == 0), stop=(t == slots - 1),
                        )
                    nb_sb = wp.tile([P, D2], bf16, tag="nb", bufs=RSG_G + 2,
                                    name=f"nb{k}")
                    nb_t[k] = nb_sb
                    nc.scalar.copy(nb_sb[:], nb_ps[:])
                    st = wp.tile([P, 4, 6], f32, tag="st", name=f"st{k}")
                    for b in range(2):
                        nc.vector.bn_stats(st[:, b, :],
                                           xs_sb[:, k, b * D : (b + 1) * D])
                        nc.vector.bn_stats(st[:, 2 + b, :],
                                           nb_sb[:, b * D : (b + 1) * D])
                        nc.vector.bn_aggr(mvg[:, 2 * gi_ + b, :],
                                          st[:, b : b + 1, :])
                        nc.vector.bn_aggr(mvg[:, 2 * gs + 2 * gi_ + b, :],
                                          st[:, 2 + b : 3 + b, :])

                # phase A': group rstd via bit-hack + Newton (all on DVE)
                ve = gw.tile([P, 4 * gs], f32, tag="ve", name=f"ve{k0}")
                nc.vector.tensor_scalar(
                    out=ve[:, : 2 * gs], in0=mvg[:, : 2 * gs, 1],
                    scalar1=1e-5, scalar2=None, op0=AOP.add)
                for gi_ in range(gs):
                    k = k0 + gi_
                    sl = slice(2 * gs + 2 * gi_, 2 * gs + 2 * gi_ + 2)
                    nc.vector.tensor_scalar(
                        out=ve[:, sl], in0=mvg[:, sl, 1],
                        scalar1=invd2[:, k : k + 1], scalar2=1e-5,
                        op0=AOP.mult, op1=AOP.add)
                rsg = gw.tile([P, 4 * gs], f32, tag="rsg", name=f"rsg{k0}")
                t0_ = gw.tile([P, 4 * gs], f32, tag="tn", name=f"tn{k0}")
                nc.vector.tensor_scalar(
                    out=rsg[:].bitcast(i32), in0=ve[:].bitcast(i32),
                    scalar1=1, scalar2=None, op0=AOP.logical_shift_right)
                nc.vector.tensor_scalar(
                    out=rsg[:].bitcast(i32), in0=rsg[:].bitcast(i32),
                    scalar1=-1, scalar2=0x5F3759DF,
                    op0=AOP.mult, op1=AOP.add)
                for _ in range(2):
                    nc.vector.tensor_tensor(t0_[:], rsg[:], rsg[:], op=AOP.mult)
                    nc.vector.tensor_tensor(t0_[:], t0_[:], ve[:], op=AOP.mult)
                    nc.vector.tensor_scalar(
                        out=t0_[:], in0=t0_[:], scalar1=-0.5, scalar2=1.5,
                        op0=AOP.mult, op1=AOP.add)
                    nc.vector.tensor_tensor(rsg[:], rsg[:], t0_[:], op=AOP.mult)
                # bias_g = -mean * rs
                biag = gw.tile([P, 4 * gs], f32, tag="biag", name=f"biag{k0}")
                nc.vector.scalar_tensor_tensor(
                    out=biag[:], in0=mvg[:, :, 0], scalar=-1.0, in1=rsg[:],
                    op0=AOP.mult, op1=AOP.mult)
                # nb scale/bias folded with 1/deg
                scn = gw.tile([P, 2 * gs], f32, tag="scn", name=f"scn{k0}")
                bin_ = gw.tile([P, 2 * gs], f32, tag="bin", name=f"bin{k0}")
                for gi_ in range(gs):
                    k = k0 + gi_
                    sl = slice(2 * gs + 2 * gi_, 2 * gs + 2 * gi_ + 2)
                    ol = slice(2 * gi_, 2 * gi_ + 2)
                    nc.vector.tensor_scalar(
                        out=scn[:, ol], in0=rsg[:, sl],
                        scalar1=invd[:, k : k + 1], scalar2=None, op0=AOP.mult)
                    nc.vector.tensor_scalar(
                        out=bin_[:, ol], in0=biag[:, sl],
                        scalar1=invd[:, k : k + 1], scalar2=None, op0=AOP.mult)

                # phase B: LN-folded evacuations, transposes, MLP, residual
                for gi_ in range(gs):
                    k = k0 + gi_
                    nb_sb = nb_t[k]
                    hx = wp.tile([P, D2], bf16, tag="hx", name=f"hx{k}")
                    hn = wp.tile([P, D2], bf16, tag="hn", name=f"hn{k}")
                    for b in range(2):
                        nc.scalar.activation(
                            hx[:, b * D : (b + 1) * D],
                            xs_sb[:, k, b * D : (b + 1) * D], AF.Identity,
                            bias=biag[:, 2 * gi_ + b : 2 * gi_ + b + 1],
                            scale=rsg[:, 2 * gi_ + b : 2 * gi_ + b + 1])
                        nc.scalar.activation(
                            hn[:, b * D : (b + 1) * D],
                            nb_sb[:, b * D : (b + 1) * D], AF.Identity,
                            bias=bin_[:, 2 * gi_ + b : 2 * gi_ + b + 1],
                            scale=scn[:, 2 * gi_ + b : 2 * gi_ + b + 1])

                    tp = trps.tile([P, 4 * P], bf16, space="PSUM", tag="tr",
                                   name=f"tr{k}")
                    for b in range(2):
                        for kt, srct in ((0, hx), (1, hn)):
                            nc.tensor.transpose(
                                tp[:, (2 * kt + b) * P : (2 * kt + b + 1) * P],
                                srct[:, b * D : (b + 1) * D], identb[:])
                    hTcc = wp.tile([P, 4 * P], bf16, tag="hT", name=f"hT{k}")
                    nc.scalar.copy(hTcc[:], tp[:])

                    m1 = mm1ps.tile([P, 2, D2], f32, space="PSUM", tag="m1",
                                    name=f"m1_{k}")
                    for jt in range(2):
                        for kt in range(2):
                            nc.tensor.matmul(
                                m1[:, jt, :], lhsT=w1s[kt][jt][:],
                                rhs=hTcc[:, 2 * kt * P : 2 * (kt + 1) * P],
                                start=(kt == 0), stop=(kt == 1),
                            )
                    gsb = wp.tile([P, 2, D2], bf16, tag="gc", name=f"gc{k}")
                    for jt in range(2):
                        nc.scalar.activation(
                            gsb[:, jt, :], m1[:, jt, :], AF.Gelu,
                            bias=b1e_sb[:, jt : jt + 1], scale=1.0)

                    m2 = mm2ps.tile([P, D2], f32, space="PSUM", tag="m2",
                                    name=f"m2_{k}")
                    for b in range(2):
                        for jt in range(2):
                            nc.tensor.matmul(
                                m2[:, b * D : (b + 1) * D],
                                lhsT=gsb[:, jt, b * D : (b + 1) * D],
                                rhs=w2t[jt][:],
                                start=(jt == 0), stop=False,
                            )
                        nc.tensor.matmul(
                            m2[:, b * D : (b + 1) * D],
                            lhsT=ones1[:], rhs=b2r_sb[:],
                            start=False, stop=True,
                        )
                    y_sb = wp.tile([P, D2], bf16, tag="y", name=f"y{k}")
                    nc.vector.tensor_tensor(y_sb[:], m2[:], xs_sb[:, k, :],
                                            op=AOP.add)
                    nc.sync.dma_start(d_y[k * P : (k + 1) * P, :], y_sb[:])
    nc.compile()
    return nc


def kernel(x, edge_src, edge_dst, degree, sn_g, sn_b, nn_g, nn_b, W1, b1, W2, b2):
    from concourse.bass_utils import run_bass_kernel_spmd

    x = np.asarray(x)
    Bb, N, D = x.shape
    assert Bb == 2 and D == P, (Bb, N, D)

    struct, percore, shared, tids, N = _prep(x, edge_src, edge_dst, degree)

    key = (struct["NTC"], struct["T0"], struct["T1"],
           struct["NA"], struct["NB"])
    if key not in _CACHE:
        _CACHE.clear()
        _CACHE[key] = _build(struct)
    nc = _CACHE[key]

    W1f = np.asarray(W1, dtype=np.float32)
    b1f = np.asarray(b1, dtype=np.float32).ravel()
    shared_map = dict(
        xpa=shared["xpa"],
        xpb=shared["xpb"],
        w1=np.ascontiguousarray(W1f),
        w2b=np.ascontiguousarray(np.asarray(W2, np.float32)).astype(
            ml_dtypes.bfloat16),
        b1r=np.ascontiguousarray(b1f.reshape(2, P).T),
        b2r=np.asarray(b2, np.float32).reshape(1, P).astype(ml_dtypes.bfloat16),
        gx=np.asarray(sn_g, np.float32).reshape(P, 1),
        gn=np.asarray(nn_g, np.float32).reshape(P, 1),
        bx=np.asarray(sn_b, np.float32).reshape(P, 1),
        bn=np.asarray(nn_b, np.float32).reshape(P, 1),
    )

    in_maps = []
    for c in range(NCORES):
        m = dict(shared_map)
        m["xs"] = np.ascontiguousarray(percore["xs"][c])
        m["idx"] = np.ascontiguousarray(percore["idx"][c])
        m["dlb"] = np.ascontiguousarray(percore["dlb"][c])
        m["deg"] = np.ascontiguousarray(percore["deg"][c])
        in_maps.append(m)

    # the axon-tunneled device occasionally reports
    # NRT_EXEC_UNIT_UNRECOVERABLE on the first attempt; a retry recovers it
    last_exc = None
    for _attempt in range(3):
        try:
            res = run_bass_kernel_spmd(nc, in_maps, core_ids=list(range(NCORES)))
            break
        except Exception as e:  # noqa: BLE001
            last_exc = e
    else:
        raise last_exc

    y = np.empty((Bb, N, P), dtype=np.float32)
    NTC = struct["NTC"]
    for c in range(NCORES):
        yc = np.asarray(res.results[c]["y"]).astype(np.float32)
        for k in range(NTC):
            t = tids[c][k]
            n0 = t * P
            n1 = min(n0 + P, N)
            if n1 <= n0:
                continue
            y[0, n0:n1, :] = yc[k * P : k * P + (n1 - n0), :P]
            y[1, n0:n1, :] = yc[k * P : k * P + (n1 - n0), P:]
    return y


# revision 13
# speedup vs baseline: 1.0134x; 1.0134x over previous
"""Trainium2 Bass kernel for nn_MeshGraphBlock (GNN message-passing block).

Computes, for x:[B,N,D], edges (src,dst):[E], degree:[N]:
    neighbor = scatter_add(x[:, src, :] -> dst) / clip(degree, 1)
    h  = concat(LN(x; sn_g, sn_b), LN(neighbor; nn_g, nn_b))   # [B,N,2D]
    h  = gelu_erf(h @ W1 + b1)                                  # [B,N,2D]
    y  = x + h @ W2 + b2                                        # [B,N,D]

Strategy (8 NeuronCores, SPMD, one compiled program; only data differs):
 - Destination-node tiles (128 nodes) dealt to cores by sorted round-robin.
 - Host pre-sorts edges by dst, packs x as a bf16 [N,2D] gather table (two
   int16-indexed halves), emits per-core gather indices with -1 padding
   (padded descriptors are skipped by the SWDGE ucode, so gather DMA pays
   for real edges only).
 - On device: dma_gather edge messages (512B rows); one-hot scatter
   matrices generated with a single broadcast-compare DVE op per position;
   scatter-add via PE matmuls into PSUM; LayerNorm mean/rstd from
   bn_stats on PSUM + a bit-hack Newton rsqrt on DVE; LN applied for free
   inside the Act-engine PSUM evacuation (per-partition scale/bias).
 - MLP runs fully in bf16 (PE transposes, both matmuls, residual read),
   activation table never swaps (gelu/identity/copy only).
"""

import math

import numpy as np
import ml_dtypes

P = 128
NCORES = 8
SPLIT = 32768           # int16 gather-index limit
CHUNK = 1536            # max idxs per dma_gather call (SWDGE ring)
SCRATCH = 32768         # dynamic_dma_scratch_size -> 2048-descriptor ring
GBUFS = 5               # gather-buffer pool depth (first GBUFS positions
                        # gather their padding too, to init SBUF)
SBUFS = 3
RSG_G = 6               # positions per rstd group

_CACHE = {}


def _prep(x, edge_src, edge_dst, degree):
    """Host-side sharding. Returns (structure, per-core inputs, tids)."""
    Bb, N, D = x.shape
    es = np.asarray(edge_src).astype(np.int64).ravel()
    ed = np.asarray(edge_dst).astype(np.int64).ravel()
    deg = np.asarray(degree).astype(np.float32).ravel()

    ntiles = math.ceil(N / P)
    ntiles_pad = math.ceil(ntiles / NCORES) * NCORES
    NTC = ntiles_pad // NCORES

    order = np.argsort(ed, kind="stable")
    ed_s = ed[order]
    es_s = es[order]
    bounds = np.searchsorted(ed_s, np.arange(ntiles_pad + 1) * P)

    counts = bounds[1:] - bounds[:-1]
    ranked = np.argsort(-counts, kind="stable")
    tids = [[0] * NTC for _ in range(NCORES)]
    for i, t in enumerate(ranked):
        tids[i % NCORES][i // NCORES] = int(t)

    # per (core, pos): G0 (src < SPLIT) and G1 index/dst-local streams
    gi = {}
    dli = {}
    for c in range(NCORES):
        for k in range(NTC):
            t = tids[c][k]
            a, b = bounds[t], bounds[t + 1]
            srcs = es_s[a:b]
            dloc = (ed_s[a:b] - t * P).astype(np.float32)
            m0 = srcs < SPLIT
            gi[c, k, 0] = srcs[m0].astype(np.int16)
            gi[c, k, 1] = (srcs[~m0] - SPLIT).astype(np.int16)
            dli[c, k, 0] = dloc[m0]
            dli[c, k, 1] = dloc[~m0]

    nmax = {(k, g): max(len(gi[c, k, g]) for c in range(NCORES))
            for k in range(NTC) for g in (0, 1)}
    T0 = [math.ceil(nmax[k, 0] / P) for k in range(NTC)]
    T1 = [math.ceil(nmax[k, 1] / P) for k in range(NTC)]
    TS = [t0 + t1 for t0, t1 in zip(T0, T1)]
    dl_off = np.concatenate([[0], np.cumsum(TS)])
    TTOT = int(dl_off[-1])

    idx_flat = np.zeros((NCORES, TTOT * P), dtype=np.int16)
    dl_flat = np.full((NCORES, TTOT * P), -1.0, dtype=np.float32)
    calls = []  # (pos, grp, slot_off, nt, idx_off, num) uniform across cores
    for k in range(NTC):
        for g, nt in ((0, T0[k]), (1, T1[k])):
            if nt == 0:
                continue
            so = int(dl_off[k]) + (T0[k] if g else 0)
            o = so * P
            for c in range(NCORES):
                ii = gi[c, k, g]
                dd = dli[c, k, g]
                idx_flat[c, o : o + len(ii)] = ii
                dl_flat[c, o : o + len(dd)] = dd
            num = math.ceil(nmax[k, g] / 16) * 16  # ucode reads 16-groups
            if k < GBUFS:
                # prime the gather pool: gather the padding too (idx 0)
                for c in range(NCORES):
                    idx_flat[c, o + len(gi[c, k, g]) : o + nt * P] = 0
                num = nt * P
            done = 0
            while done < num:
                n = min(num - done, CHUNK)
                calls.append((k, g, so + done // P, math.ceil(n / P),
                              o + done, n))
                done += n

    idx_wrapped = np.stack(
        [np.tile(idx_flat[c].reshape(-1, 16).T, (8, 1)) for c in range(NCORES)]
    )
    dlb = np.stack(
        [np.ascontiguousarray(dl_flat[c].reshape(TTOT, P).T)
         for c in range(NCORES)]
    )  # [NCORES, 128, TTOT] float32

    deg_r = np.ones((NCORES, P, NTC), dtype=np.float32)
    xs = np.zeros((NCORES, NTC * P, 2 * D), dtype=ml_dtypes.bfloat16)
    xf = np.asarray(x, dtype=np.float32)
    xcat = np.concatenate([xf[0], xf[1]], axis=1).astype(ml_dtypes.bfloat16)
    for c in range(NCORES):
        for k in range(NTC):
            t = tids[c][k]
            n0 = t * P
            n1 = min(n0 + P, N)
            if n1 <= n0:
                continue
            deg_r[c, : n1 - n0, k] = deg[n0:n1]
            xs[c, k * P : k * P + (n1 - n0), :] = xcat[n0:n1]

    # gather tables (rows %128-padded so whole-tile loads stay in range)
    xpa = np.ascontiguousarray(xcat[:SPLIT])
    nb_rows = math.ceil((ntiles_pad * P - SPLIT) / P) * P
    xpb = np.zeros((nb_rows, 2 * D), dtype=ml_dtypes.bfloat16)
    xpb[: N - SPLIT] = xcat[SPLIT:]

    struct = dict(NTC=NTC, T0=tuple(T0), T1=tuple(T1), TS=tuple(TS),
                  TTOT=TTOT, calls=tuple(calls),
                  dl_off=tuple(int(v) for v in dl_off),
                  NA=xpa.shape[0], NB=xpb.shape[0], D=D, Bb=Bb)
    percore = dict(idx=idx_wrapped, dlb=dlb, deg=deg_r, xs=xs)
    shared = dict(xpa=xpa, xpb=xpb)
    return struct, percore, shared, tids, N


def _build(struct):
    import concourse.bacc as bacc
    import concourse.tile as tile
    from concourse import mybir
    from concourse.masks import make_identity

    NTC, T0, T1, TS = struct["NTC"], struct["T0"], struct["T1"], struct["TS"]
    TTOT = struct["TTOT"]
    dl_off = struct["dl_off"]
    calls = struct["calls"]
    D = struct["D"]
    D2 = 2 * D
    TSMAX = max(TS)
    f32, bf16, i16 = mybir.dt.float32, mybir.dt.bfloat16, mybir.dt.int16
    i32 = mybir.dt.int32
    AOP = mybir.AluOpType
    AF = mybir.ActivationFunctionType

    calls_by_pos = {}
    for (k, g, so, nt, io, num) in calls:
        calls_by_pos.setdefault(k, []).append((g, so, nt, io, num))

    nc = bacc.Bacc("TRN2", target_bir_lowering=False, debug=False,
                   dynamic_dma_scratch_size=SCRATCH)
    d_xpa = nc.dram_tensor("xpa", [struct["NA"], D2], bf16, kind="ExternalInput")
    d_xpb = nc.dram_tensor("xpb", [struct["NB"], D2], bf16, kind="ExternalInput")
    d_xs = nc.dram_tensor("xs", [NTC * P, D2], bf16, kind="ExternalInput")
    d_idx = nc.dram_tensor("idx", [P, TTOT * 8], i16, kind="ExternalInput")
    d_dlb = nc.dram_tensor("dlb", [P, TTOT], f32, kind="ExternalInput")
    d_deg = nc.dram_tensor("deg", [P, NTC], f32, kind="ExternalInput")
    d_w1 = nc.dram_tensor("w1", [D2, D2], f32, kind="ExternalInput")
    d_w2 = nc.dram_tensor("w2b", [D2, D], bf16, kind="ExternalInput")
    d_b1 = nc.dram_tensor("b1r", [P, 2], f32, kind="ExternalInput")
    d_b2 = nc.dram_tensor("b2r", [1, P], bf16, kind="ExternalInput")
    d_gx = nc.dram_tensor("gx", [P, 1], f32, kind="ExternalInput")
    d_gn = nc.dram_tensor("gn", [P, 1], f32, kind="ExternalInput")
    d_bx = nc.dram_tensor("bx", [P, 1], f32, kind="ExternalInput")
    d_bn = nc.dram_tensor("bn", [P, 1], f32, kind="ExternalInput")
    d_y = nc.dram_tensor("y", [NTC * P, D2], bf16, kind="ExternalOutput")

    with tile.TileContext(nc) as tc:
        with (
            tc.tile_pool(name="const", bufs=1) as cp,
            tc.tile_pool(name="gath", bufs=GBUFS) as gpool,
            tc.tile_pool(name="sel", bufs=SBUFS) as spool,
            tc.tile_pool(name="work", bufs=3) as wp,
            tc.tile_pool(name="grp", bufs=2) as gw,
            tc.tile_pool(name="nbps", bufs=2, space="PSUM") as nbps,
            tc.tile_pool(name="trps", bufs=2, space="PSUM") as trps,
            tc.tile_pool(name="mm1ps", bufs=2, space="PSUM") as mm1ps,
            tc.tile_pool(name="mm2ps", bufs=2, space="PSUM") as mm2ps,
        ):
            # ---- one-time constants ----
            idx_sb = cp.tile([P, TTOT * 8], i16)
            nc.sync.dma_start(idx_sb[:], d_idx.ap())
            dlb_sb = cp.tile([P, TTOT], f32)
            nc.sync.dma_start(dlb_sb[:], d_dlb.ap())
            deg_sb = cp.tile([P, NTC], f32)
            nc.sync.dma_start(deg_sb[:], d_deg.ap())
            invd = cp.tile([P, NTC], f32)
            nc.vector.tensor_scalar_max(invd[:], deg_sb[:], 1.0)
            nc.vector.reciprocal(invd[:], invd[:])
            invd2 = cp.tile([P, NTC], f32)
            nc.vector.tensor_tensor(invd2[:], invd[:], invd[:], op=AOP.mult)

            identb = cp.tile([P, P], bf16)
            make_identity(nc, identb[:])
            iota3 = cp.tile([P, TSMAX, P], bf16)
            nc.gpsimd.iota(iota3[:], pattern=[[0, TSMAX], [1, P]], base=0,
                           channel_multiplier=0,
                           allow_small_or_imprecise_dtypes=True)

            gx_sb = cp.tile([P, 1], f32); nc.sync.dma_start(gx_sb[:], d_gx.ap())
            gn_sb = cp.tile([P, 1], f32); nc.sync.dma_start(gn_sb[:], d_gn.ap())
            bx_sb = cp.tile([P, 1], f32); nc.sync.dma_start(bx_sb[:], d_bx.ap())
            bn_sb = cp.tile([P, 1], f32); nc.sync.dma_start(bn_sb[:], d_bn.ap())
            b1r_sb = cp.tile([P, 2], f32); nc.sync.dma_start(b1r_sb[:], d_b1.ap())
            b2r_sb = cp.tile([1, P], bf16); nc.sync.dma_start(b2r_sb[:], d_b2.ap())
            ones1 = cp.tile([1, P], bf16)
            nc.vector.memset(ones1[:], 1.0)

            # W1 f32 tiles (for b1 fold), gamma-scaled bf16 copies, W2 bf16
            w1t = [[cp.tile([P, P], f32, name=f"w1t{kt}{jt}") for jt in range(2)]
                   for kt in range(2)]
            w1s = [[cp.tile([P, P], bf16, name=f"w1s{kt}{jt}") for jt in range(2)]
                   for kt in range(2)]
            gam = [gx_sb, gn_sb]
            for kt in range(2):
                for jt in range(2):
                    nc.sync.dma_start(
                        w1t[kt][jt][:],
                        d_w1[kt * P : (kt + 1) * P, jt * P : (jt + 1) * P],
                    )
                    nc.vector.tensor_scalar_mul(
                        w1s[kt][jt][:], w1t[kt][jt][:], gam[kt][:]
                    )
            w2t = [cp.tile([P, P], bf16, name=f"w2t{kt}") for kt in range(2)]
            for kt in range(2):
                nc.sync.dma_start(w2t[kt][:], d_w2[kt * P : (kt + 1) * P, :])

            # b1_eff = b1 + beta_cat @ W1  (per-partition layout [128, jt])
            bet = [bx_sb, bn_sb]
            b1b_ps = mm1ps.tile([P, 2], f32, space="PSUM", tag="m1")
            for jt in range(2):
                for kt in range(2):
                    nc.tensor.matmul(
                        b1b_ps[:, jt : jt + 1], lhsT=w1t[kt][jt][:],
                        rhs=bet[kt][:], start=(kt == 0), stop=(kt == 1),
                    )
            b1e_sb = cp.tile([P, 2], f32)
            nc.vector.tensor_add(b1e_sb[:], b1b_ps[:], b1r_sb[:])

            # resident x slices: [128, NTC, 2D] bf16
            xs_sb = cp.tile([P, NTC, D2], bf16)
            for k in range(NTC):
                nc.sync.dma_start(xs_sb[:, k, :], d_xs[k * P : (k + 1) * P, :])

            # ---- main loop: groups of RSG_G positions ----
            for k0 in range(0, NTC, RSG_G):
                gs = min(RSG_G, NTC - k0)
                nb_t = {}
                # stats layout: x entries [2k], nb entries [2G + 2k]
                mvg = gw.tile([P, 4 * gs, 2], f32, tag="mvg", name=f"mvg{k0}")

                # phase A: gather, scatter-add, stats
                for gi_ in range(gs):
                    k = k0 + gi_
                    slots = TS[k]
                    g = gpool.tile([P, TSMAX, D2], bf16, tag="g", name=f"g{k}")
                    for (grp, so, nt, io, num) in calls_by_pos.get(k, []):
                        src_t = d_xpa if grp == 0 else d_xpb
                        so_l = so - dl_off[k]
                        nc.gpsimd.dma_gather(
                            g[:, so_l : so_l + nt, :], src_t.ap(),
                            idx_sb[:, io // 16 : (io + nt * P) // 16],
                            num, num, D2, single_packet=False,
                        )
                    S = spool.tile([P, TSMAX, P], bf16, tag="S", name=f"S{k}")
                    for t in range(slots):
                        nc.vector.tensor_scalar(
                            out=S[:, t, :], in0=iota3[:, 0, :],
                            scalar1=dlb_sb[:, dl_off[k] + t : dl_off[k] + t + 1],
                            scalar2=None, op0=AOP.is_equal)
                    nb_ps = nbps.tile([P, D2], f32, space="PSUM", tag="nbp",
                                      name=f"nbp{k}")
                    for t in range(slots):
                        nc.tensor.matmul(
                            nb_ps[:], lhsT=S[:, t, :], rhs=g[:, t, :],
                            start=(t == 0), stop=(t == slots - 1),
                        )
                    nb_sb = wp.tile([P, D2], bf16, tag="nb", bufs=RSG_G + 2,
                                    name=f"nb{k}")
                    nb_t[k] = nb_sb
                    nc.scalar.copy(nb_sb[:], nb_ps[:])
                    st = wp.tile([P, 4, 6], f32, tag="st", name=f"st{k}")
                    for b in range(2):
                        nc.vector.bn_stats(st[:, b, :],
                                           xs_sb[:, k, b * D : (b + 1) * D])
                        nc.vector.bn_stats(st[:, 2 + b, :],
                                           nb_sb[:, b * D : (b + 1) * D])
                        nc.vector.bn_aggr(mvg[:, 2 * gi_ + b, :],
                                          st[:, b : b + 1, :])
                        nc.vector.bn_aggr(mvg[:, 2 * gs + 2 * gi_ + b, :],
                                          st[:, 2 + b : 3 + b, :])

                # phase A': group rstd via bit-hack + Newton (all on DVE)
                ve = gw.tile([P, 4 * gs], f32, tag="ve", name=f"ve{k0}")
                nc.vector.tensor_scalar(
                    out=ve[:, : 2 * gs], in0=mvg[:, : 2 * gs, 1],
                    scalar1=1e-5, scalar2=None, op0=AOP.add)
                for gi_ in range(gs):
                    k = k0 + gi_
                    sl = slice(2 * gs + 2 * gi_, 2 * gs + 2 * gi_ + 2)
                    nc.vector.tensor_scalar(
                        out=ve[:, sl], in0=mvg[:, sl, 1],
                        scalar1=invd2[:, k : k + 1], scalar2=1e-5,
                        op0=AOP.mult, op1=AOP.add)
                rsg = gw.tile([P, 4 * gs], f32, tag="rsg", name=f"rsg{k0}")
                t0_ = gw.tile([P, 4 * gs], f32, tag="tn", name=f"tn{k0}")
                nc.vector.tensor_scalar(
                    out=rsg[:].bitcast(i32), in0=ve[:].bitcast(i32),
                    scalar1=1, scalar2=None, op0=AOP.logical_shift_right)
                nc.vector.tensor_scalar(
                    out=rsg[:].bitcast(i32), in0=rsg[:].bitcast(i32),
                    scalar1=-1, scalar2=0x5F3759DF,
                    op0=AOP.mult, op1=AOP.add)
                for _ in range(2):
                    nc.vector.tensor_tensor(t0_[:], rsg[:], rsg[:], op=AOP.mult)
                    nc.vector.tensor_tensor(t0_[:], t0_[:], ve[:], op=AOP.mult)
                    nc.vector.tensor_scalar(
                        out=t0_[:], in0=t0_[:], scalar1=-0.5, scalar2=1.5,
                        op0=AOP.mult, op1=AOP.add)
                    nc.vector.tensor_tensor(rsg[:], rsg[:], t0_[:], op=AOP.mult)
                # bias_g = -mean * rs
                biag = gw.tile([P, 4 * gs], f32, tag="biag", name=f"biag{k0}")
                nc.vector.scalar_tensor_tensor(
                    out=biag[:], in0=mvg[:, :, 0], scalar=-1.0, in1=rsg[:],
                    op0=AOP.mult, op1=AOP.mult)
                # nb scale/bias folded with 1/deg
                scn = gw.tile([P, 2 * gs], f32, tag="scn", name=f"scn{k0}")
                bin_ = gw.tile([P, 2 * gs], f32, tag="bin", name=f"bin{k0}")
                for gi_ in range(gs):
                    k = k0 + gi_
                    sl = slice(2 * gs + 2 * gi_, 2 * gs + 2 * gi_ + 2)
                    ol = slice(2 * gi_, 2 * gi_ + 2)
                    nc.vector.tensor_scalar(
                        out=scn[:, ol], in0=rsg[:, sl],
                        scalar1=invd[:, k : k + 1], scalar2=None, op0=AOP.mult)
                    nc.vector.tensor_scalar(
                        out=bin_[:, ol], in0=biag[:, sl],
                        scalar1=invd[:, k : k + 1], scalar2=None, op0=AOP.mult)

                # phase B: LN-folded evacuations, transposes, MLP, residual
                for gi_ in range(gs):
                    k = k0 + gi_
                    nb_sb = nb_t[k]
                    hx = wp.tile([P, D2], bf16, tag="hx", name=f"hx{k}")
                    hn = wp.tile([P, D2], bf16, tag="hn", name=f"hn{k}")
                    for b in range(2):
                        nc.scalar.activation(
                            hx[:, b * D : (b + 1) * D],
                            xs_sb[:, k, b * D : (b + 1) * D], AF.Identity,
                            bias=biag[:, 2 * gi_ + b : 2 * gi_ + b + 1],
                            scale=rsg[:, 2 * gi_ + b : 2 * gi_ + b + 1])
                        nc.scalar.activation(
                            hn[:, b * D : (b + 1) * D],
                            nb_sb[:, b * D : (b + 1) * D], AF.Identity,
                            bias=bin_[:, 2 * gi_ + b : 2 * gi_ + b + 1],
                            scale=scn[:, 2 * gi_ + b : 2 * gi_ + b + 1])

                    tp = trps.tile([P, 4 * P], bf16, space="PSUM", tag="tr",
                                   name=f"tr{k}")
                    for b in range(2):
                        for kt, srct in ((0, hx), (1, hn)):
                            nc.tensor.transpose(
                                tp[:, (2 * kt + b) * P : (2 * kt + b + 1) * P],
                                srct[:, b * D : (b + 1) * D], identb[:])
                    hTcc = wp.tile([P, 4 * P], bf16, tag="hT", name=f"hT{k}")
                    nc.scalar.copy(hTcc[:], tp[:])

                    m1 = mm1ps.tile([P, 2, D2], f32, space="PSUM", tag="m1",
                                    name=f"m1_{k}")
                    for jt in range(2):
                        for kt in range(2):
                            nc.tensor.matmul(
                                m1[:, jt, :], lhsT=w1s[kt][jt][:],
                                rhs=hTcc[:, 2 * kt * P : 2 * (kt + 1) * P],
                                start=(kt == 0), stop=(kt == 1),
                            )
                    gsb = wp.tile([P, 2, D2], bf16, tag="gc", name=f"gc{k}")
                    for jt in range(2):
                        nc.scalar.activation(
                            gsb[:, jt, :], m1[:, jt, :], AF.Gelu,
                            bias=b1e_sb[:, jt : jt + 1], scale=1.0)

                    m2 = mm2ps.tile([P, D2], f32, space="PSUM", tag="m2",
                                    name=f"m2_{k}")
                    for b in range(2):
                        for jt in range(2):
                            nc.tensor.matmul(
                                m2[:, b * D : (b + 1) * D],
                                lhsT=gsb[:, jt, b * D : (b + 1) * D],
                                rhs=w2t[jt][:],
                                start=(jt == 0), stop=False,
                            )
                        nc.tensor.matmul(
                            m2[:, b * D : (b + 1) * D],
                            lhsT=ones1[:], rhs=b2r_sb[:],
                            start=False, stop=True,
                        )
                    y_sb = wp.tile([P, D2], bf16, tag="y", name=f"y{k}")
                    nc.vector.tensor_tensor(y_sb[:], m2[:], xs_sb[:, k, :],
                                            op=AOP.add)
                    nc.sync.dma_start(d_y[k * P : (k + 1) * P, :], y_sb[:])
    nc.compile()
    return nc


def kernel(x, edge_src, edge_dst, degree, sn_g, sn_b, nn_g, nn_b, W1, b1, W2, b2):
    from concourse.bass_utils import run_bass_kernel_spmd

    x = np.asarray(x)
    Bb, N, D = x.shape
    assert Bb == 2 and D == P, (Bb, N, D)

    struct, percore, shared, tids, N = _prep(x, edge_src, edge_dst, degree)

    key = (struct["NTC"], struct["T0"], struct["T1"],
           struct["NA"], struct["NB"])
    if key not in _CACHE:
        _CACHE.clear()
        _CACHE[key] = _build(struct)
    nc = _CACHE[key]

    W1f = np.asarray(W1, dtype=np.float32)
    b1f = np.asarray(b1, dtype=np.float32).ravel()
    shared_map = dict(
        xpa=shared["xpa"],
        xpb=shared["xpb"],
        w1=np.ascontiguousarray(W1f),
        w2b=np.ascontiguousarray(np.asarray(W2, np.float32)).astype(
            ml_dtypes.bfloat16),
        b1r=np.ascontiguousarray(b1f.reshape(2, P).T),
        b2r=np.asarray(b2, np.float32).reshape(1, P).astype(ml_dtypes.bfloat16),
        gx=np.asarray(sn_g, np.float32).reshape(P, 1),
        gn=np.asarray(nn_g, np.float32).reshape(P, 1),
        bx=np.asarray(sn_b, np.float32).reshape(P, 1),
        bn=np.asarray(nn_b, np.float32).reshape(P, 1),
    )

    in_maps = []
    for c in range(NCORES):
        m = dict(shared_map)
        m["xs"] = np.ascontiguousarray(percore["xs"][c])
        m["idx"] = np.ascontiguousarray(percore["idx"][c])
        m["dlb"] = np.ascontiguousarray(percore["dlb"][c])
        m["deg"] = np.ascontiguousarray(percore["deg"][c])
        in_maps.append(m)

    # the axon-tunneled device occasionally reports
    # NRT_EXEC_UNIT_UNRECOVERABLE on the first attempt; a retry recovers it
    last_exc = None
    for _attempt in range(3):
        try:
            res = run_bass_kernel_spmd(nc, in_maps, core_ids=list(range(NCORES)))
            break
        except Exception as e:  # noqa: BLE001
            last_exc = e
    else:
        raise last_exc

    y = np.empty((Bb, N, P), dtype=np.float32)
    NTC = struct["NTC"]
    for c in range(NCORES):
        yc = np.asarray(res.results[c]["y"]).astype(np.float32)
        for k in range(NTC):
            t = tids[c][k]
            n0 = t * P
            n1 = min(n0 + P, N)
            if n1 <= n0:
                continue
            y[0, n0:n1, :] = yc[k * P : k * P + (n1 - n0), :P]
            y[1, n0:n1, :] = yc[k * P : k * P + (n1 - n0), P:]
    return y


# revision 23
# speedup vs baseline: 1.0166x; 1.0031x over previous
"""Trainium2 Bass kernel for nn_MeshGraphBlock (GNN message-passing block).

Computes, for x:[B,N,D], edges (src,dst):[E], degree:[N]:
    neighbor = scatter_add(x[:, src, :] -> dst) / clip(degree, 1)
    h  = concat(LN(x; sn_g, sn_b), LN(neighbor; nn_g, nn_b))   # [B,N,2D]
    h  = gelu_erf(h @ W1 + b1)                                  # [B,N,2D]
    y  = x + h @ W2 + b2                                        # [B,N,D]

Strategy (8 NeuronCores, SPMD, one compiled program; only data differs):
 - Destination-node tiles (128 nodes) dealt to cores by sorted round-robin.
 - Host pre-sorts edges by dst, packs x as a bf16 [N,2D] gather table (two
   int16-indexed halves), emits per-core gather indices with -1 padding
   (padded descriptors are skipped by the SWDGE ucode, so gather DMA pays
   for real edges only).
 - On device: dma_gather edge messages (512B rows); one-hot scatter
   matrices generated with a single broadcast-compare DVE op per position;
   scatter-add via PE matmuls into PSUM; LayerNorm mean/rstd from
   bn_stats on PSUM + a bit-hack Newton rsqrt on DVE; LN applied for free
   inside the Act-engine PSUM evacuation (per-partition scale/bias).
 - MLP runs fully in bf16 (PE transposes, both matmuls, residual read),
   activation table never swaps (gelu/identity/copy only).
"""

import math

import numpy as np
import ml_dtypes

P = 128
NCORES = 8
SPLIT = 32768           # int16 gather-index limit
CHUNK = 1536            # max idxs per dma_gather call (SWDGE ring)
SCRATCH = 32768         # dynamic_dma_scratch_size -> 2048-descriptor ring
GBUFS = 6               # gather-buffer pool depth (first GBUFS positions
                        # gather their padding too, to init SBUF)
SBUFS = 4
RSG_G = 6               # positions per rstd group

_CACHE = {}


def _prep(x, edge_src, edge_dst, degree):
    """Host-side sharding. Returns (structure, per-core inputs, tids)."""
    Bb, N, D = x.shape
    es = np.asarray(edge_src).astype(np.int64).ravel()
    ed = np.asarray(edge_dst).astype(np.int64).ravel()
    deg = np.asarray(degree).astype(np.float32).ravel()

    ntiles = math.ceil(N / P)
    ntiles_pad = math.ceil(ntiles / NCORES) * NCORES
    NTC = ntiles_pad // NCORES

    order = np.argsort(ed, kind="stable")
    ed_s = ed[order]
    es_s = es[order]
    bounds = np.searchsorted(ed_s, np.arange(ntiles_pad + 1) * P)

    counts = bounds[1:] - bounds[:-1]
    ranked = np.argsort(-counts, kind="stable")
    tids = [[0] * NTC for _ in range(NCORES)]
    for i, t in enumerate(ranked):
        tids[i % NCORES][i // NCORES] = int(t)

    # per (core, pos): G0 (src < SPLIT) and G1 index/dst-local streams
    gi = {}
    dli = {}
    for c in range(NCORES):
        for k in range(NTC):
            t = tids[c][k]
            a, b = bounds[t], bounds[t + 1]
            srcs = es_s[a:b]
            dloc = (ed_s[a:b] - t * P).astype(np.float32)
            m0 = srcs < SPLIT
            gi[c, k, 0] = srcs[m0].astype(np.int16)
            gi[c, k, 1] = (srcs[~m0] - SPLIT).astype(np.int16)
            dli[c, k, 0] = dloc[m0]
            dli[c, k, 1] = dloc[~m0]

    nmax = {(k, g): max(len(gi[c, k, g]) for c in range(NCORES))
            for k in range(NTC) for g in (0, 1)}
    T0 = [math.ceil(nmax[k, 0] / P) for k in range(NTC)]
    T1 = [math.ceil(nmax[k, 1] / P) for k in range(NTC)]
    TS = [t0 + t1 for t0, t1 in zip(T0, T1)]
    dl_off = np.concatenate([[0], np.cumsum(TS)])
    TTOT = int(dl_off[-1])

    idx_flat = np.zeros((NCORES, TTOT * P), dtype=np.int16)
    dl_flat = np.full((NCORES, TTOT * P), -1.0, dtype=np.float32)
    calls = []  # (pos, grp, slot_off, nt, idx_off, num) uniform across cores
    for k in range(NTC):
        for g, nt in ((0, T0[k]), (1, T1[k])):
            if nt == 0:
                continue
            so = int(dl_off[k]) + (T0[k] if g else 0)
            o = so * P
            for c in range(NCORES):
                ii = gi[c, k, g]
                dd = dli[c, k, g]
                idx_flat[c, o : o + len(ii)] = ii
                dl_flat[c, o : o + len(dd)] = dd
            num = math.ceil(nmax[k, g] / 16) * 16  # ucode reads 16-groups
            done = 0
            while done < num:
                n = min(num - done, CHUNK)
                calls.append((k, g, so + done // P, math.ceil(n / P),
                              o + done, n))
                done += n

    idx_wrapped = np.stack(
        [np.tile(idx_flat[c].reshape(-1, 16).T, (8, 1)) for c in range(NCORES)]
    )
    dlb = np.stack(
        [np.ascontiguousarray(dl_flat[c].reshape(TTOT, P).T)
         for c in range(NCORES)]
    )  # [NCORES, 128, TTOT] float32

    deg_r = np.ones((NCORES, P, NTC), dtype=np.float32)
    xs = np.zeros((NCORES, NTC * P, 2 * D), dtype=ml_dtypes.bfloat16)
    xf = np.asarray(x, dtype=np.float32)
    xcat = np.concatenate([xf[0], xf[1]], axis=1).astype(ml_dtypes.bfloat16)
    for c in range(NCORES):
        for k in range(NTC):
            t = tids[c][k]
            n0 = t * P
            n1 = min(n0 + P, N)
            if n1 <= n0:
                continue
            deg_r[c, : n1 - n0, k] = deg[n0:n1]
            xs[c, k * P : k * P + (n1 - n0), :] = xcat[n0:n1]

    # gather tables (rows %128-padded so whole-tile loads stay in range)
    xpa = np.ascontiguousarray(xcat[:SPLIT])
    nb_rows = math.ceil((ntiles_pad * P - SPLIT) / P) * P
    xpb = np.zeros((nb_rows, 2 * D), dtype=ml_dtypes.bfloat16)
    xpb[: N - SPLIT] = xcat[SPLIT:]

    struct = dict(NTC=NTC, T0=tuple(T0), T1=tuple(T1), TS=tuple(TS),
                  TTOT=TTOT, calls=tuple(calls),
                  dl_off=tuple(int(v) for v in dl_off),
                  NA=xpa.shape[0], NB=xpb.shape[0], D=D, Bb=Bb)
    percore = dict(idx=idx_wrapped, dlb=dlb,
                   deg=np.repeat(deg_r, 2, axis=-1), xs=xs)
    shared = dict(xpa=xpa, xpb=xpb)
    return struct, percore, shared, tids, N


def _build(struct):
    import concourse.bacc as bacc
    import concourse.tile as tile
    from concourse import mybir
    from concourse.masks import make_identity

    NTC, T0, T1, TS = struct["NTC"], struct["T0"], struct["T1"], struct["TS"]
    TTOT = struct["TTOT"]
    dl_off = struct["dl_off"]
    calls = struct["calls"]
    D = struct["D"]
    D2 = 2 * D
    TSMAX = max(TS)
    f32, bf16, i16 = mybir.dt.float32, mybir.dt.bfloat16, mybir.dt.int16
    i32 = mybir.dt.int32
    AOP = mybir.AluOpType
    AF = mybir.ActivationFunctionType

    calls_by_pos = {}
    for (k, g, so, nt, io, num) in calls:
        calls_by_pos.setdefault(k, []).append((g, so, nt, io, num))

    nc = bacc.Bacc("TRN2", target_bir_lowering=False, debug=False,
                   dynamic_dma_scratch_size=SCRATCH)
    d_xpa = nc.dram_tensor("xpa", [struct["NA"], D2], bf16, kind="ExternalInput")
    d_xpb = nc.dram_tensor("xpb", [struct["NB"], D2], bf16, kind="ExternalInput")
    d_xs = nc.dram_tensor("xs", [NTC * P, D2], bf16, kind="ExternalInput")
    d_idx = nc.dram_tensor("idx", [P, TTOT * 8], i16, kind="ExternalInput")
    d_dlb = nc.dram_tensor("dlb", [P, TTOT], f32, kind="ExternalInput")
    d_deg = nc.dram_tensor("deg", [P, 2 * NTC], f32, kind="ExternalInput")
    d_w1 = nc.dram_tensor("w1", [D2, D2], f32, kind="ExternalInput")
    d_w2 = nc.dram_tensor("w2b", [D2, D], bf16, kind="ExternalInput")
    d_b1 = nc.dram_tensor("b1r", [P, 2], f32, kind="ExternalInput")
    d_b2 = nc.dram_tensor("b2r", [1, P], bf16, kind="ExternalInput")
    d_gx = nc.dram_tensor("gx", [P, 1], f32, kind="ExternalInput")
    d_gn = nc.dram_tensor("gn", [P, 1], f32, kind="ExternalInput")
    d_bx = nc.dram_tensor("bx", [P, 1], f32, kind="ExternalInput")
    d_bn = nc.dram_tensor("bn", [P, 1], f32, kind="ExternalInput")
    d_y = nc.dram_tensor("y", [NTC * P, D2], bf16, kind="ExternalOutput")

    with tile.TileContext(nc) as tc:
        with (
            tc.tile_pool(name="const", bufs=1) as cp,
            tc.tile_pool(name="sel", bufs=SBUFS) as spool,
            tc.tile_pool(name="work", bufs=3) as wp,
            tc.tile_pool(name="grp", bufs=2) as gw,
            tc.tile_pool(name="nbps", bufs=2, space="PSUM") as nbps,
            tc.tile_pool(name="trps", bufs=2, space="PSUM") as trps,
            tc.tile_pool(name="mm1ps", bufs=2, space="PSUM") as mm1ps,
            tc.tile_pool(name="mm2ps", bufs=2, space="PSUM") as mm2ps,
        ):
            # ---- one-time constants ----
            idx_sb = cp.tile([P, TTOT * 8], i16)
            nc.sync.dma_start(idx_sb[:], d_idx.ap())
            dlb_sb = cp.tile([P, TTOT], f32)
            nc.sync.dma_start(dlb_sb[:], d_dlb.ap())
            deg_sb = cp.tile([P, 2 * NTC], f32)
            nc.sync.dma_start(deg_sb[:], d_deg.ap())
            invd = cp.tile([P, 2 * NTC], f32)
            nc.vector.tensor_scalar_max(invd[:], deg_sb[:], 1.0)
            nc.vector.reciprocal(invd[:], invd[:])
            invd2 = cp.tile([P, 2 * NTC], f32)
            nc.vector.tensor_tensor(invd2[:], invd[:], invd[:], op=AOP.mult)

            identb = cp.tile([P, P], bf16)
            make_identity(nc, identb[:])
            iota3 = cp.tile([P, TSMAX, P], bf16)
            nc.gpsimd.iota(iota3[:], pattern=[[0, TSMAX], [1, P]], base=0,
                           channel_multiplier=0,
                           allow_small_or_imprecise_dtypes=True)

            gx_sb = cp.tile([P, 1], f32); nc.sync.dma_start(gx_sb[:], d_gx.ap())
            gn_sb = cp.tile([P, 1], f32); nc.sync.dma_start(gn_sb[:], d_gn.ap())
            bx_sb = cp.tile([P, 1], f32); nc.sync.dma_start(bx_sb[:], d_bx.ap())
            bn_sb = cp.tile([P, 1], f32); nc.sync.dma_start(bn_sb[:], d_bn.ap())
            b1r_sb = cp.tile([P, 2], f32); nc.sync.dma_start(b1r_sb[:], d_b1.ap())
            b2r_sb = cp.tile([1, P], bf16); nc.sync.dma_start(b2r_sb[:], d_b2.ap())
            ones1 = cp.tile([1, P], bf16)
            nc.vector.memset(ones1[:], 1.0)

            # W1 f32 tiles (for b1 fold), gamma-scaled bf16 copies, W2 bf16
            w1t = [[cp.tile([P, P], f32, name=f"w1t{kt}{jt}") for jt in range(2)]
                   for kt in range(2)]
            w1s = [[cp.tile([P, P], bf16, name=f"w1s{kt}{jt}") for jt in range(2)]
                   for kt in range(2)]
            gam = [gx_sb, gn_sb]
            for kt in range(2):
                for jt in range(2):
                    nc.sync.dma_start(
                        w1t[kt][jt][:],
                        d_w1[kt * P : (kt + 1) * P, jt * P : (jt + 1) * P],
                    )
                    nc.vector.tensor_scalar_mul(
                        w1s[kt][jt][:], w1t[kt][jt][:], gam[kt][:]
                    )
            w2t = [cp.tile([P, P], bf16, name=f"w2t{kt}") for kt in range(2)]
            for kt in range(2):
                nc.sync.dma_start(w2t[kt][:], d_w2[kt * P : (kt + 1) * P, :])

            # b1_eff = b1 + beta_cat @ W1  (per-partition layout [128, jt])
            bet = [bx_sb, bn_sb]
            b1b_ps = mm1ps.tile([P, 2], f32, space="PSUM", tag="m1")
            for jt in range(2):
                for kt in range(2):
                    nc.tensor.matmul(
                        b1b_ps[:, jt : jt + 1], lhsT=w1t[kt][jt][:],
                        rhs=bet[kt][:], start=(kt == 0), stop=(kt == 1),
                    )
            b1e_sb = cp.tile([P, 2], f32)
            nc.vector.tensor_add(b1e_sb[:], b1b_ps[:], b1r_sb[:])

            # resident x slices: [128, NTC, 2D] bf16
            xs_sb = cp.tile([P, NTC, D2], bf16)
            for k in range(NTC):
                nc.sync.dma_start(xs_sb[:, k, :], d_xs[k * P : (k + 1) * P, :])

            # manual gather ring: fixed buffers, zeroed once (tail slots of
            # later rounds keep stale-but-finite data; round 1 must not read
            # virgin SBUF, which can hold NaN bit patterns)
            g_ring = []
            for i in range(GBUFS):
                gt = cp.tile([P, TSMAX, D2], bf16, name=f"gring{i}")
                nc.vector.memset(gt[:], 0.0)
                g_ring.append(gt)

            def rsqrt_newton(ve, rs, tn):
                # rs = 1/sqrt(ve), via bit hack + 2 Newton steps (DVE only)
                nc.vector.tensor_scalar(
                    out=rs.bitcast(i32), in0=ve.bitcast(i32),
                    scalar1=1, scalar2=None, op0=AOP.logical_shift_right)
                nc.vector.tensor_scalar(
                    out=rs.bitcast(i32), in0=rs.bitcast(i32),
                    scalar1=-1, scalar2=0x5F3759DF,
                    op0=AOP.mult, op1=AOP.add)
                for _ in range(2):
                    nc.vector.tensor_tensor(tn, rs, rs, op=AOP.mult)
                    nc.vector.tensor_tensor(tn, tn, ve, op=AOP.mult)
                    nc.vector.tensor_scalar(
                        out=tn, in0=tn, scalar1=-0.5, scalar2=1.5,
                        op0=AOP.mult, op1=AOP.add)
                    nc.vector.tensor_tensor(rs, rs, tn, op=AOP.mult)

            # x-side LN scalars for all positions, one batched chain
            stx = cp.tile([P, 2 * NTC, 6], f32)
            for k in range(NTC):
                for b in range(2):
                    nc.vector.bn_stats(stx[:, 2 * k + b, :],
                                       xs_sb[:, k, b * D : (b + 1) * D])
            smx = cp.tile([P, 2 * NTC], f32)
            vex = cp.tile([P, 2 * NTC], f32)
            cvx = cp.tile([P, 2 * NTC], f32)
            rsgx = cp.tile([P, 2 * NTC], f32)
            tnx = cp.tile([P, 2 * NTC], f32)
            biagx = cp.tile([P, 2 * NTC], f32)
            nc.vector.tensor_tensor(smx[:], stx[:, :, 1], stx[:, :, 4],
                                    op=AOP.add)
            nc.vector.tensor_tensor(vex[:], stx[:, :, 1], stx[:, :, 4],
                                    op=AOP.subtract)
            nc.vector.scalar_tensor_tensor(
                out=vex[:], in0=vex[:], scalar=0.25, in1=vex[:],
                op0=AOP.mult, op1=AOP.mult)
            nc.vector.tensor_tensor(cvx[:], stx[:, :, 2], stx[:, :, 5],
                                    op=AOP.add)
            nc.vector.scalar_tensor_tensor(
                out=vex[:], in0=cvx[:], scalar=1.0 / P, in1=vex[:],
                op0=AOP.mult, op1=AOP.add)
            nc.vector.tensor_scalar(out=vex[:], in0=vex[:], scalar1=1e-5,
                                    scalar2=None, op0=AOP.add)
            rsqrt_newton(vex[:], rsgx[:], tnx[:])
            nc.vector.scalar_tensor_tensor(
                out=biagx[:], in0=smx[:], scalar=-0.5, in1=rsgx[:],
                op0=AOP.mult, op1=AOP.mult)

            # ---- main loop: groups of positions (small tail groups) ----
            bounds_g = []
            k0 = 0
            while k0 < NTC:
                rem = NTC - k0
                if rem > RSG_G + 4 or rem <= RSG_G:
                    gsz = min(RSG_G, rem)
                else:
                    gsz = rem - rem // 2
                bounds_g.append((k0, gsz))
                k0 += gsz
            for (k0, gs) in bounds_g:
                nb_t = {}
                stg = gw.tile([P, 2 * RSG_G, 6], f32, tag="stg",
                              name=f"stg{k0}")

                # phase A: gather, scatter-add, stats
                for gi_ in range(gs):
                    k = k0 + gi_
                    slots = TS[k]
                    g = g_ring[k % GBUFS]
                    for (grp, so, nt, io, num) in calls_by_pos.get(k, []):
                        src_t = d_xpa if grp == 0 else d_xpb
                        so_l = so - dl_off[k]
                        nc.gpsimd.dma_gather(
                            g[:, so_l : so_l + nt, :], src_t.ap(),
                            idx_sb[:, io // 16 : io // 16 + (num + 15) // 16],
                            num, num, D2, single_packet=False,
                        )
                    S = spool.tile([P, TSMAX, P], bf16, tag="S", name=f"S{k}")
                    for t in range(slots):
                        nc.vector.tensor_scalar(
                            out=S[:, t, :], in0=iota3[:, 0, :],
                            scalar1=dlb_sb[:, dl_off[k] + t : dl_off[k] + t + 1],
                            scalar2=None, op0=AOP.is_equal)
                    nb_ps = nbps.tile([P, D2], f32, space="PSUM", tag="nbp",
                                      name=f"nbp{k}")
                    for t in range(slots):
                        nc.tensor.matmul(
                            nb_ps[:], lhsT=S[:, t, :], rhs=g[:, t, :],
                            start=(t == 0), stop=(t == slots - 1),
                        )
                    nb_sb = wp.tile([P, D2], bf16, tag="nb", bufs=RSG_G + 2,
                                    name=f"nb{k}")
                    nb_t[k] = nb_sb
                    nc.scalar.copy(nb_sb[:], nb_ps[:])
                    for b in range(2):
                        nc.vector.bn_stats(stg[:, 2 * gi_ + b, :],
                                           nb_sb[:, b * D : (b + 1) * D])

                # phase A': nb-side LN scalars (batched bit-hack rsqrt)
                n2 = 2 * gs
                ofs = 2 * k0
                smn = gw.tile([P, 2 * RSG_G], f32, tag="smn", name=f"smn{k0}")
                ven = gw.tile([P, 2 * RSG_G], f32, tag="ven", name=f"ven{k0}")
                cvn = gw.tile([P, 2 * RSG_G], f32, tag="cvn", name=f"cvn{k0}")
                rsn = gw.tile([P, 2 * RSG_G], f32, tag="rsn", name=f"rsn{k0}")
                tnn = gw.tile([P, 2 * RSG_G], f32, tag="tnn", name=f"tnn{k0}")
                scn = gw.tile([P, 2 * RSG_G], f32, tag="scn", name=f"scn{k0}")
                bin_ = gw.tile([P, 2 * RSG_G], f32, tag="bin", name=f"bin{k0}")
                nc.vector.tensor_tensor(smn[:, :n2], stg[:, :n2, 1],
                                        stg[:, :n2, 4], op=AOP.add)
                nc.vector.tensor_tensor(ven[:, :n2], stg[:, :n2, 1],
                                        stg[:, :n2, 4], op=AOP.subtract)
                nc.vector.scalar_tensor_tensor(
                    out=ven[:, :n2], in0=ven[:, :n2], scalar=0.25,
                    in1=ven[:, :n2], op0=AOP.mult, op1=AOP.mult)
                nc.vector.tensor_tensor(cvn[:, :n2], stg[:, :n2, 2],
                                        stg[:, :n2, 5], op=AOP.add)
                nc.vector.scalar_tensor_tensor(
                    out=ven[:, :n2], in0=cvn[:, :n2], scalar=1.0 / P,
                    in1=ven[:, :n2], op0=AOP.mult, op1=AOP.add)
                # ve = var_psum * invd^2 + eps
                nc.vector.tensor_tensor(ven[:, :n2], ven[:, :n2],
                                        invd2[:, ofs : ofs + n2], op=AOP.mult)
                nc.vector.tensor_scalar(out=ven[:, :n2], in0=ven[:, :n2],
                                        scalar1=1e-5, scalar2=None,
                                        op0=AOP.add)
                rsqrt_newton(ven[:, :n2], rsn[:, :n2], tnn[:, :n2])
                # scale = invd*rs ; bias = -(mean_psum*invd)*rs
                nc.vector.tensor_tensor(scn[:, :n2], rsn[:, :n2],
                                        invd[:, ofs : ofs + n2], op=AOP.mult)
                nc.vector.scalar_tensor_tensor(
                    out=bin_[:, :n2], in0=smn[:, :n2], scalar=-0.5,
                    in1=scn[:, :n2], op0=AOP.mult, op1=AOP.mult)

                # phase B: LN-folded evacuations, transposes, MLP, residual
                for gi_ in range(gs):
                    k = k0 + gi_
                    nb_sb = nb_t[k]
                    hx = wp.tile([P, D2], bf16, tag="hx", name=f"hx{k}")
                    hn = wp.tile([P, D2], bf16, tag="hn", name=f"hn{k}")
                    for b in range(2):
                        e = 2 * k + b
                        nc.scalar.activation(
                            hx[:, b * D : (b + 1) * D],
                            xs_sb[:, k, b * D : (b + 1) * D], AF.Identity,
                            bias=biagx[:, e : e + 1],
                            scale=rsgx[:, e : e + 1])
                        nc.scalar.activation(
                            hn[:, b * D : (b + 1) * D],
                            nb_sb[:, b * D : (b + 1) * D], AF.Identity,
                            bias=bin_[:, 2 * gi_ + b : 2 * gi_ + b + 1],
                            scale=scn[:, 2 * gi_ + b : 2 * gi_ + b + 1])

                    tp = trps.tile([P, 4 * P], bf16, space="PSUM", tag="tr",
                                   name=f"tr{k}")
                    for b in range(2):
                        for kt, srct in ((0, hx), (1, hn)):
                            nc.tensor.transpose(
                                tp[:, (2 * kt + b) * P : (2 * kt + b + 1) * P],
                                srct[:, b * D : (b + 1) * D], identb[:])
                    hTcc = wp.tile([P, 4 * P], bf16, tag="hT", name=f"hT{k}")
                    nc.scalar.copy(hTcc[:], tp[:])

                    m1 = mm1ps.tile([P, 2, D2], f32, space="PSUM", tag="m1",
                                    name=f"m1_{k}")
                    for jt in range(2):
                        for kt in range(2):
                            nc.tensor.matmul(
                                m1[:, jt, :], lhsT=w1s[kt][jt][:],
                                rhs=hTcc[:, 2 * kt * P : 2 * (kt + 1) * P],
                                start=(kt == 0), stop=(kt == 1),
                            )
                    gsb = wp.tile([P, 2, D2], bf16, tag="gc", name=f"gc{k}")
                    for jt in range(2):
                        nc.scalar.activation(
                            gsb[:, jt, :], m1[:, jt, :], AF.Gelu,
                            bias=b1e_sb[:, jt : jt + 1], scale=1.0)

                    m2 = mm2ps.tile([P, D2], f32, space="PSUM", tag="m2",
                                    name=f"m2_{k}")
                    for b in range(2):
                        for jt in range(2):
                            nc.tensor.matmul(
                                m2[:, b * D : (b + 1) * D],
                                lhsT=gsb[:, jt, b * D : (b + 1) * D],
                                rhs=w2t[jt][:],
                                start=(jt == 0), stop=False,
                            )
                        nc.tensor.matmul(
                            m2[:, b * D : (b + 1) * D],
                            lhsT=ones1[:], rhs=b2r_sb[:],
                            start=False, stop=True,
                        )
                    y_sb = wp.tile([P, D2], bf16, tag="y", name=f"y{k}")
                    nc.vector.tensor_tensor(y_sb[:], m2[:], xs_sb[:, k, :],
                                            op=AOP.add)
                    nc.sync.dma_start(d_y[k * P : (k + 1) * P, :], y_sb[:])
    nc.compile()
    return nc


def kernel(x, edge_src, edge_dst, degree, sn_g, sn_b, nn_g, nn_b, W1, b1, W2, b2):
    from concourse.bass_utils import run_bass_kernel_spmd

    x = np.asarray(x)
    Bb, N, D = x.shape
    assert Bb == 2 and D == P, (Bb, N, D)

    struct, percore, shared, tids, N = _prep(x, edge_src, edge_dst, degree)

    key = (struct["NTC"], struct["T0"], struct["T1"],
           struct["NA"], struct["NB"])
    if key not in _CACHE:
        _CACHE.clear()
        _CACHE[key] = _build(struct)
    nc = _CACHE[key]

    W1f = np.asarray(W1, dtype=np.float32)
    b1f = np.asarray(b1, dtype=np.float32).ravel()
    shared_map = dict(
        xpa=shared["xpa"],
        xpb=shared["xpb"],
        w1=np.ascontiguousarray(W1f),
        w2b=np.ascontiguousarray(np.asarray(W2, np.float32)).astype(
            ml_dtypes.bfloat16),
        b1r=np.ascontiguousarray(b1f.reshape(2, P).T),
        b2r=np.asarray(b2, np.float32).reshape(1, P).astype(ml_dtypes.bfloat16),
        gx=np.asarray(sn_g, np.float32).reshape(P, 1),
        gn=np.asarray(nn_g, np.float32).reshape(P, 1),
        bx=np.asarray(sn_b, np.float32).reshape(P, 1),
        bn=np.asarray(nn_b, np.float32).reshape(P, 1),
    )

    in_maps = []
    for c in range(NCORES):
        m = dict(shared_map)
        m["xs"] = np.ascontiguousarray(percore["xs"][c])
        m["idx"] = np.ascontiguousarray(percore["idx"][c])
        m["dlb"] = np.ascontiguousarray(percore["dlb"][c])
        m["deg"] = np.ascontiguousarray(percore["deg"][c])
        in_maps.append(m)

    # the axon-tunneled device occasionally reports
    # NRT_EXEC_UNIT_UNRECOVERABLE on the first attempt; a retry recovers it
    import sys
    import time
    last_exc = None
    for _attempt in range(3):
        try:
            t0 = time.time()
            res = run_bass_kernel_spmd(nc, in_maps, core_ids=list(range(NCORES)))
            print(f"[kernel] attempt {_attempt} ok in {time.time()-t0:.1f}s",
                  file=sys.stderr)
            break
        except Exception as e:  # noqa: BLE001
            print(f"[kernel] attempt {_attempt} failed after "
                  f"{time.time()-t0:.1f}s: {str(e)[:200]}", file=sys.stderr)
            last_exc = e
    else:
        raise last_exc

    y = np.empty((Bb, N, P), dtype=np.float32)
    NTC = struct["NTC"]
    for c in range(NCORES):
        yc = np.asarray(res.results[c]["y"]).astype(np.float32)
        for k in range(NTC):
            t = tids[c][k]
            n0 = t * P
            n1 = min(n0 + P, N)
            if n1 <= n0:
                continue
            y[0, n0:n1, :] = yc[k * P : k * P + (n1 - n0), :P]
            y[1, n0:n1, :] = yc[k * P : k * P + (n1 - n0), P:]
    return y


# revision 25
# speedup vs baseline: 1.1590x; 1.1401x over previous
"""Trainium2 Bass kernel for nn_MeshGraphBlock (GNN message-passing block).

Computes, for x:[B,N,D], edges (src,dst):[E], degree:[N]:
    neighbor = scatter_add(x[:, src, :] -> dst) / clip(degree, 1)
    h  = concat(LN(x; sn_g, sn_b), LN(neighbor; nn_g, nn_b))   # [B,N,2D]
    h  = gelu_erf(h @ W1 + b1)                                  # [B,N,2D]
    y  = x + h @ W2 + b2                                        # [B,N,D]

Strategy (8 NeuronCores, SPMD, one compiled program; only data differs):
 - Destination-node tiles (128 nodes) dealt to cores by sorted round-robin.
 - Host pre-sorts edges by dst, packs x as a bf16 [N,2D] gather table (two
   int16-indexed halves), emits per-core gather indices with -1 padding
   (padded descriptors are skipped by the SWDGE ucode, so gather DMA pays
   for real edges only).
 - On device: dma_gather edge messages (512B rows); one-hot scatter
   matrices generated with a single broadcast-compare DVE op per position;
   scatter-add via PE matmuls into PSUM; LayerNorm mean/rstd from
   bn_stats on PSUM + a bit-hack Newton rsqrt on DVE; LN applied for free
   inside the Act-engine PSUM evacuation (per-partition scale/bias).
 - MLP runs fully in bf16 (PE transposes, both matmuls, residual read),
   activation table never swaps (gelu/identity/copy only).
"""

import math

import numpy as np
import ml_dtypes

P = 128
NCORES = 8
SPLIT = 32768           # int16 gather-index limit
CHUNK = 1536            # max idxs per dma_gather call (SWDGE ring)
SCRATCH = 32768         # dynamic_dma_scratch_size -> 2048-descriptor ring
GBUFS = 6               # gather-buffer pool depth (first GBUFS positions
                        # gather their padding too, to init SBUF)
SBUFS = 4
RSG_G = 6               # positions per rstd group

_CACHE = {}


def _prep(x, edge_src, edge_dst, degree):
    """Host-side sharding. Returns (structure, per-core inputs, tids)."""
    Bb, N, D = x.shape
    es = np.asarray(edge_src).astype(np.int64).ravel()
    ed = np.asarray(edge_dst).astype(np.int64).ravel()
    deg = np.asarray(degree).astype(np.float32).ravel()

    ntiles = math.ceil(N / P)
    ntiles_pad = math.ceil(ntiles / NCORES) * NCORES
    NTC = ntiles_pad // NCORES

    order = np.argsort(ed, kind="stable")
    ed_s = ed[order]
    es_s = es[order]
    bounds = np.searchsorted(ed_s, np.arange(ntiles_pad + 1) * P)

    counts = bounds[1:] - bounds[:-1]
    ranked = np.argsort(-counts, kind="stable")
    tids = [[0] * NTC for _ in range(NCORES)]
    for i, t in enumerate(ranked):
        tids[i % NCORES][i // NCORES] = int(t)

    # per (core, pos): G0 (src < SPLIT) and G1 index/dst-local streams
    gi = {}
    dli = {}
    for c in range(NCORES):
        for k in range(NTC):
            t = tids[c][k]
            a, b = bounds[t], bounds[t + 1]
            srcs = es_s[a:b]
            dloc = (ed_s[a:b] - t * P).astype(np.float32)
            m0 = srcs < SPLIT
            gi[c, k, 0] = srcs[m0].astype(np.int16)
            gi[c, k, 1] = (srcs[~m0] - SPLIT).astype(np.int16)
            dli[c, k, 0] = dloc[m0]
            dli[c, k, 1] = dloc[~m0]

    nmax = {(k, g): max(len(gi[c, k, g]) for c in range(NCORES))
            for k in range(NTC) for g in (0, 1)}
    T0 = [math.ceil(nmax[k, 0] / P) for k in range(NTC)]
    T1 = [math.ceil(nmax[k, 1] / P) for k in range(NTC)]
    TS = [t0 + t1 for t0, t1 in zip(T0, T1)]
    dl_off = np.concatenate([[0], np.cumsum(TS)])
    TTOT = int(dl_off[-1])

    idx_flat = np.zeros((NCORES, TTOT * P), dtype=np.int16)
    dl_flat = np.full((NCORES, TTOT * P), -1.0, dtype=np.float32)
    calls = []  # (pos, grp, slot_off, nt, idx_off, num) uniform across cores
    for k in range(NTC):
        for g, nt in ((0, T0[k]), (1, T1[k])):
            if nt == 0:
                continue
            so = int(dl_off[k]) + (T0[k] if g else 0)
            o = so * P
            for c in range(NCORES):
                ii = gi[c, k, g]
                dd = dli[c, k, g]
                idx_flat[c, o : o + len(ii)] = ii
                dl_flat[c, o : o + len(dd)] = dd
            num = math.ceil(nmax[k, g] / 16) * 16  # ucode reads 16-groups
            done = 0
            while done < num:
                n = min(num - done, CHUNK)
                calls.append((k, g, so + done // P, math.ceil(n / P),
                              o + done, n))
                done += n

    idx_wrapped = np.stack(
        [np.tile(idx_flat[c].reshape(-1, 16).T, (8, 1)) for c in range(NCORES)]
    )
    dlb = np.stack(
        [np.ascontiguousarray(dl_flat[c].reshape(TTOT, P).T)
         for c in range(NCORES)]
    )  # [NCORES, 128, TTOT] float32

    deg_r = np.ones((NCORES, P, NTC), dtype=np.float32)
    xs = np.zeros((NCORES, NTC * P, 2 * D), dtype=ml_dtypes.bfloat16)
    xf = np.asarray(x, dtype=np.float32)
    xcat = np.concatenate([xf[0], xf[1]], axis=1).astype(ml_dtypes.bfloat16)
    for c in range(NCORES):
        for k in range(NTC):
            t = tids[c][k]
            n0 = t * P
            n1 = min(n0 + P, N)
            if n1 <= n0:
                continue
            deg_r[c, : n1 - n0, k] = deg[n0:n1]
            xs[c, k * P : k * P + (n1 - n0), :] = xcat[n0:n1]

    # gather tables (rows %128-padded so whole-tile loads stay in range)
    xpa = np.ascontiguousarray(xcat[:SPLIT])
    nb_rows = math.ceil((ntiles_pad * P - SPLIT) / P) * P
    xpb = np.zeros((nb_rows, 2 * D), dtype=ml_dtypes.bfloat16)
    xpb[: N - SPLIT] = xcat[SPLIT:]

    struct = dict(NTC=NTC, T0=tuple(T0), T1=tuple(T1), TS=tuple(TS),
                  TTOT=TTOT, calls=tuple(calls),
                  dl_off=tuple(int(v) for v in dl_off),
                  NA=xpa.shape[0], NB=xpb.shape[0], D=D, Bb=Bb)
    percore = dict(idx=idx_wrapped, dlb=dlb,
                   deg=np.repeat(deg_r, 2, axis=-1), xs=xs)
    shared = dict(xpa=xpa, xpb=xpb)
    return struct, percore, shared, tids, N


def _build(struct):
    import concourse.bacc as bacc
    import concourse.tile as tile
    from concourse import mybir
    from concourse.masks import make_identity

    NTC, T0, T1, TS = struct["NTC"], struct["T0"], struct["T1"], struct["TS"]
    TTOT = struct["TTOT"]
    dl_off = struct["dl_off"]
    calls = struct["calls"]
    D = struct["D"]
    D2 = 2 * D
    TSMAX = max(TS)
    f32, bf16, i16 = mybir.dt.float32, mybir.dt.bfloat16, mybir.dt.int16
    i32 = mybir.dt.int32
    AOP = mybir.AluOpType
    AF = mybir.ActivationFunctionType

    calls_by_pos = {}
    for (k, g, so, nt, io, num) in calls:
        calls_by_pos.setdefault(k, []).append((g, so, nt, io, num))

    nc = bacc.Bacc("TRN2", target_bir_lowering=False, debug=False,
                   dynamic_dma_scratch_size=SCRATCH)
    d_xpa = nc.dram_tensor("xpa", [struct["NA"], D2], bf16, kind="ExternalInput")
    d_xpb = nc.dram_tensor("xpb", [struct["NB"], D2], bf16, kind="ExternalInput")
    d_xs = nc.dram_tensor("xs", [NTC * P, D2], bf16, kind="ExternalInput")
    d_idx = nc.dram_tensor("idx", [P, TTOT * 8], i16, kind="ExternalInput")
    d_dlb = nc.dram_tensor("dlb", [P, TTOT], f32, kind="ExternalInput")
    d_deg = nc.dram_tensor("deg", [P, 2 * NTC], f32, kind="ExternalInput")
    d_w1 = nc.dram_tensor("w1", [D2, D2], f32, kind="ExternalInput")
    d_w2 = nc.dram_tensor("w2b", [D2, D], bf16, kind="ExternalInput")
    d_b1 = nc.dram_tensor("b1r", [P, 2], f32, kind="ExternalInput")
    d_b2 = nc.dram_tensor("b2r", [1, P], bf16, kind="ExternalInput")
    d_gx = nc.dram_tensor("gx", [P, 1], f32, kind="ExternalInput")
    d_gn = nc.dram_tensor("gn", [P, 1], f32, kind="ExternalInput")
    d_bx = nc.dram_tensor("bx", [P, 1], f32, kind="ExternalInput")
    d_bn = nc.dram_tensor("bn", [P, 1], f32, kind="ExternalInput")
    d_y = nc.dram_tensor("y", [NTC * P, D2], bf16, kind="ExternalOutput")

    with tile.TileContext(nc) as tc:
        with (
            tc.tile_pool(name="const", bufs=1) as cp,
            tc.tile_pool(name="sel", bufs=SBUFS) as spool,
            tc.tile_pool(name="work", bufs=3) as wp,
            tc.tile_pool(name="grp", bufs=2) as gw,
            tc.tile_pool(name="nbps", bufs=2, space="PSUM") as nbps,
            tc.tile_pool(name="trps", bufs=2, space="PSUM") as trps,
            tc.tile_pool(name="mm1ps", bufs=2, space="PSUM") as mm1ps,
            tc.tile_pool(name="mm2ps", bufs=2, space="PSUM") as mm2ps,
        ):
            # ---- one-time constants ----
            idx_sb = cp.tile([P, TTOT * 8], i16)
            nc.sync.dma_start(idx_sb[:], d_idx.ap())
            dlb_sb = cp.tile([P, TTOT], f32)
            nc.sync.dma_start(dlb_sb[:], d_dlb.ap())
            deg_sb = cp.tile([P, 2 * NTC], f32)
            nc.sync.dma_start(deg_sb[:], d_deg.ap())
            invd = cp.tile([P, 2 * NTC], f32)
            nc.vector.tensor_scalar_max(invd[:], deg_sb[:], 1.0)
            nc.vector.reciprocal(invd[:], invd[:])
            invd2 = cp.tile([P, 2 * NTC], f32)
            nc.vector.tensor_tensor(invd2[:], invd[:], invd[:], op=AOP.mult)

            identb = cp.tile([P, P], bf16)
            make_identity(nc, identb[:])
            iota3 = cp.tile([P, TSMAX, P], bf16)
            nc.gpsimd.iota(iota3[:], pattern=[[0, TSMAX], [1, P]], base=0,
                           channel_multiplier=0,
                           allow_small_or_imprecise_dtypes=True)

            gx_sb = cp.tile([P, 1], f32); nc.sync.dma_start(gx_sb[:], d_gx.ap())
            gn_sb = cp.tile([P, 1], f32); nc.sync.dma_start(gn_sb[:], d_gn.ap())
            bx_sb = cp.tile([P, 1], f32); nc.sync.dma_start(bx_sb[:], d_bx.ap())
            bn_sb = cp.tile([P, 1], f32); nc.sync.dma_start(bn_sb[:], d_bn.ap())
            b1r_sb = cp.tile([P, 2], f32); nc.sync.dma_start(b1r_sb[:], d_b1.ap())
            b2r_sb = cp.tile([1, P], bf16); nc.sync.dma_start(b2r_sb[:], d_b2.ap())
            ones1 = cp.tile([1, P], bf16)
            nc.vector.memset(ones1[:], 1.0)

            # W1 f32 tiles (for b1 fold), gamma-scaled bf16 copies, W2 bf16
            w1t = [[cp.tile([P, P], f32, name=f"w1t{kt}{jt}") for jt in range(2)]
                   for kt in range(2)]
            w1s = [[cp.tile([P, P], bf16, name=f"w1s{kt}{jt}") for jt in range(2)]
                   for kt in range(2)]
            gam = [gx_sb, gn_sb]
            for kt in range(2):
                for jt in range(2):
                    nc.sync.dma_start(
                        w1t[kt][jt][:],
                        d_w1[kt * P : (kt + 1) * P, jt * P : (jt + 1) * P],
                    )
                    nc.vector.tensor_scalar_mul(
                        w1s[kt][jt][:], w1t[kt][jt][:], gam[kt][:]
                    )
            w2t = [cp.tile([P, P], bf16, name=f"w2t{kt}") for kt in range(2)]
            for kt in range(2):
                nc.sync.dma_start(w2t[kt][:], d_w2[kt * P : (kt + 1) * P, :])

            # b1_eff = b1 + beta_cat @ W1  (per-partition layout [128, jt])
            bet = [bx_sb, bn_sb]
            b1b_ps = mm1ps.tile([P, 2], f32, space="PSUM", tag="m1")
            for jt in range(2):
                for kt in range(2):
                    nc.tensor.matmul(
                        b1b_ps[:, jt : jt + 1], lhsT=w1t[kt][jt][:],
                        rhs=bet[kt][:], start=(kt == 0), stop=(kt == 1),
                    )
            b1e_sb = cp.tile([P, 2], f32)
            nc.vector.tensor_add(b1e_sb[:], b1b_ps[:], b1r_sb[:])

            # resident x slices: [128, NTC, 2D] bf16
            xs_sb = cp.tile([P, NTC, D2], bf16)
            for k in range(NTC):
                nc.sync.dma_start(xs_sb[:, k, :], d_xs[k * P : (k + 1) * P, :])

            # manual gather ring: fixed buffers, zeroed once (tail slots of
            # later rounds keep stale-but-finite data; round 1 must not read
            # virgin SBUF, which can hold NaN bit patterns)
            g_ring = []
            for i in range(GBUFS):
                gt = cp.tile([P, TSMAX, D2], bf16, name=f"gring{i}")
                nc.vector.memset(gt[:], 0.0)
                g_ring.append(gt)

            def rsqrt_newton(ve, rs, tn):
                # rs = 1/sqrt(ve), via bit hack + 2 Newton steps (DVE only)
                nc.vector.tensor_scalar(
                    out=rs.bitcast(i32), in0=ve.bitcast(i32),
                    scalar1=1, scalar2=None, op0=AOP.logical_shift_right)
                nc.vector.tensor_scalar(
                    out=rs.bitcast(i32), in0=rs.bitcast(i32),
                    scalar1=-1, scalar2=0x5F3759DF,
                    op0=AOP.mult, op1=AOP.add)
                for _ in range(2):
                    nc.vector.tensor_tensor(tn, rs, rs, op=AOP.mult)
                    nc.vector.tensor_tensor(tn, tn, ve, op=AOP.mult)
                    nc.vector.tensor_scalar(
                        out=tn, in0=tn, scalar1=-0.5, scalar2=1.5,
                        op0=AOP.mult, op1=AOP.add)
                    nc.vector.tensor_tensor(rs, rs, tn, op=AOP.mult)

            # ---- main loop: groups of positions (small tail groups) ----
            bounds_g = []
            k0 = 0
            while k0 < NTC:
                rem = NTC - k0
                if rem > RSG_G + 4 or rem <= RSG_G:
                    gsz = min(RSG_G, rem)
                else:
                    gsz = rem - rem // 2
                bounds_g.append((k0, gsz))
                k0 += gsz
            for (k0, gs) in bounds_g:
                nb_t = {}
                # stats layout: x entries [2gi+b], nb entries [2G + 2gi+b]
                stg = gw.tile([P, 4 * RSG_G, 6], f32, tag="stg",
                              name=f"stg{k0}")

                # phase A: gather, scatter-add, stats
                for gi_ in range(gs):
                    k = k0 + gi_
                    slots = TS[k]
                    g = g_ring[k % GBUFS]
                    for (grp, so, nt, io, num) in calls_by_pos.get(k, []):
                        src_t = d_xpa if grp == 0 else d_xpb
                        so_l = so - dl_off[k]
                        nc.gpsimd.dma_gather(
                            g[:, so_l : so_l + nt, :], src_t.ap(),
                            idx_sb[:, io // 16 : io // 16 + (num + 15) // 16],
                            num, num, D2, single_packet=False,
                        )
                    S = spool.tile([P, TSMAX, P], bf16, tag="S", name=f"S{k}")
                    for t in range(slots):
                        nc.vector.tensor_scalar(
                            out=S[:, t, :], in0=iota3[:, 0, :],
                            scalar1=dlb_sb[:, dl_off[k] + t : dl_off[k] + t + 1],
                            scalar2=None, op0=AOP.is_equal)
                    nb_ps = nbps.tile([P, D2], f32, space="PSUM", tag="nbp",
                                      name=f"nbp{k}")
                    for t in range(slots):
                        nc.tensor.matmul(
                            nb_ps[:], lhsT=S[:, t, :], rhs=g[:, t, :],
                            start=(t == 0), stop=(t == slots - 1),
                        )
                    nb_sb = wp.tile([P, D2], bf16, tag="nb", bufs=RSG_G + 2,
                                    name=f"nb{k}")
                    nb_t[k] = nb_sb
                    with tc.high_priority():
                        nc.scalar.copy(nb_sb[:], nb_ps[:])
                    for b in range(2):
                        nc.vector.bn_stats(stg[:, 2 * gi_ + b, :],
                                           xs_sb[:, k, b * D : (b + 1) * D])
                        nc.vector.bn_stats(stg[:, 2 * gs + 2 * gi_ + b, :],
                                           nb_sb[:, b * D : (b + 1) * D])

                # phase A': LN scalars for the group (batched rsqrt chain)
                n2 = 2 * gs
                n4 = 4 * gs
                ofs = 2 * k0
                smn = gw.tile([P, 4 * RSG_G], f32, tag="smn", name=f"smn{k0}")
                ven = gw.tile([P, 4 * RSG_G], f32, tag="ven", name=f"ven{k0}")
                cvn = gw.tile([P, 4 * RSG_G], f32, tag="cvn", name=f"cvn{k0}")
                rsn = gw.tile([P, 4 * RSG_G], f32, tag="rsn", name=f"rsn{k0}")
                tnn = gw.tile([P, 4 * RSG_G], f32, tag="tnn", name=f"tnn{k0}")
                bxg = gw.tile([P, 2 * RSG_G], f32, tag="bxg", name=f"bxg{k0}")
                scn = gw.tile([P, 2 * RSG_G], f32, tag="scn", name=f"scn{k0}")
                bin_ = gw.tile([P, 2 * RSG_G], f32, tag="bin", name=f"bin{k0}")
                nc.vector.tensor_tensor(smn[:, :n4], stg[:, :n4, 1],
                                        stg[:, :n4, 4], op=AOP.add)
                nc.vector.tensor_tensor(ven[:, :n4], stg[:, :n4, 1],
                                        stg[:, :n4, 4], op=AOP.subtract)
                nc.vector.scalar_tensor_tensor(
                    out=ven[:, :n4], in0=ven[:, :n4], scalar=0.25,
                    in1=ven[:, :n4], op0=AOP.mult, op1=AOP.mult)
                nc.vector.tensor_tensor(cvn[:, :n4], stg[:, :n4, 2],
                                        stg[:, :n4, 5], op=AOP.add)
                nc.vector.scalar_tensor_tensor(
                    out=ven[:, :n4], in0=cvn[:, :n4], scalar=1.0 / P,
                    in1=ven[:, :n4], op0=AOP.mult, op1=AOP.add)
                # nb entries: ve *= invd^2  (stats are of the raw psum sums)
                nc.vector.tensor_tensor(ven[:, n2:n4], ven[:, n2:n4],
                                        invd2[:, ofs : ofs + n2], op=AOP.mult)
                nc.vector.tensor_scalar(out=ven[:, :n4], in0=ven[:, :n4],
                                        scalar1=1e-5, scalar2=None,
                                        op0=AOP.add)
                rsqrt_newton(ven[:, :n4], rsn[:, :n4], tnn[:, :n4])
                # x: scale = rs, bias = -mean*rs ; nb additionally folds 1/deg
                nc.vector.scalar_tensor_tensor(
                    out=bxg[:, :n2], in0=smn[:, :n2], scalar=-0.5,
                    in1=rsn[:, :n2], op0=AOP.mult, op1=AOP.mult)
                nc.vector.tensor_tensor(scn[:, :n2], rsn[:, n2:n4],
                                        invd[:, ofs : ofs + n2], op=AOP.mult)
                nc.vector.scalar_tensor_tensor(
                    out=bin_[:, :n2], in0=smn[:, n2:n4], scalar=-0.5,
                    in1=scn[:, :n2], op0=AOP.mult, op1=AOP.mult)

                # phase B: LN-folded evacuations, transposes, MLP, residual
                for gi_ in range(gs):
                    k = k0 + gi_
                    nb_sb = nb_t[k]
                    hx = wp.tile([P, D2], bf16, tag="hx", name=f"hx{k}")
                    hn = wp.tile([P, D2], bf16, tag="hn", name=f"hn{k}")
                    for b in range(2):
                        e = 2 * gi_ + b
                        nc.scalar.activation(
                            hx[:, b * D : (b + 1) * D],
                            xs_sb[:, k, b * D : (b + 1) * D], AF.Identity,
                            bias=bxg[:, e : e + 1],
                            scale=rsn[:, e : e + 1])
                        nc.scalar.activation(
                            hn[:, b * D : (b + 1) * D],
                            nb_sb[:, b * D : (b + 1) * D], AF.Identity,
                            bias=bin_[:, 2 * gi_ + b : 2 * gi_ + b + 1],
                            scale=scn[:, 2 * gi_ + b : 2 * gi_ + b + 1])

                    tp = trps.tile([P, 4 * P], bf16, space="PSUM", tag="tr",
                                   name=f"tr{k}")
                    for b in range(2):
                        for kt, srct in ((0, hx), (1, hn)):
                            nc.tensor.transpose(
                                tp[:, (2 * kt + b) * P : (2 * kt + b + 1) * P],
                                srct[:, b * D : (b + 1) * D], identb[:])
                    hTcc = wp.tile([P, 4 * P], bf16, tag="hT", name=f"hT{k}")
                    nc.scalar.copy(hTcc[:], tp[:])

                    m1 = mm1ps.tile([P, 2, D2], f32, space="PSUM", tag="m1",
                                    name=f"m1_{k}")
                    for jt in range(2):
                        for kt in range(2):
                            nc.tensor.matmul(
                                m1[:, jt, :], lhsT=w1s[kt][jt][:],
                                rhs=hTcc[:, 2 * kt * P : 2 * (kt + 1) * P],
                                start=(kt == 0), stop=(kt == 1),
                            )
                    gsb = wp.tile([P, 2, D2], bf16, tag="gc", name=f"gc{k}")
                    for jt in range(2):
                        nc.scalar.activation(
                            gsb[:, jt, :], m1[:, jt, :], AF.Gelu,
                            bias=b1e_sb[:, jt : jt + 1], scale=1.0)

                    m2 = mm2ps.tile([P, D2], f32, space="PSUM", tag="m2",
                                    name=f"m2_{k}")
                    for b in range(2):
                        for jt in range(2):
                            nc.tensor.matmul(
                                m2[:, b * D : (b + 1) * D],
                                lhsT=gsb[:, jt, b * D : (b + 1) * D],
                                rhs=w2t[jt][:],
                                start=(jt == 0), stop=False,
                            )
                        nc.tensor.matmul(
                            m2[:, b * D : (b + 1) * D],
                            lhsT=ones1[:], rhs=b2r_sb[:],
                            start=False, stop=True,
                        )
                    y_sb = wp.tile([P, D2], bf16, tag="y", name=f"y{k}")
                    nc.vector.tensor_tensor(y_sb[:], m2[:], xs_sb[:, k, :],
                                            op=AOP.add)
                    nc.sync.dma_start(d_y[k * P : (k + 1) * P, :], y_sb[:])
    nc.compile()
    return nc


def kernel(x, edge_src, edge_dst, degree, sn_g, sn_b, nn_g, nn_b, W1, b1, W2, b2):
    from concourse.bass_utils import run_bass_kernel_spmd

    x = np.asarray(x)
    Bb, N, D = x.shape
    assert Bb == 2 and D == P, (Bb, N, D)

    struct, percore, shared, tids, N = _prep(x, edge_src, edge_dst, degree)

    key = (struct["NTC"], struct["T0"], struct["T1"],
           struct["NA"], struct["NB"])
    if key not in _CACHE:
        _CACHE.clear()
        _CACHE[key] = _build(struct)
    nc = _CACHE[key]

    W1f = np.asarray(W1, dtype=np.float32)
    b1f = np.asarray(b1, dtype=np.float32).ravel()
    shared_map = dict(
        xpa=shared["xpa"],
        xpb=shared["xpb"],
        w1=np.ascontiguousarray(W1f),
        w2b=np.ascontiguousarray(np.asarray(W2, np.float32)).astype(
            ml_dtypes.bfloat16),
        b1r=np.ascontiguousarray(b1f.reshape(2, P).T),
        b2r=np.asarray(b2, np.float32).reshape(1, P).astype(ml_dtypes.bfloat16),
        gx=np.asarray(sn_g, np.float32).reshape(P, 1),
        gn=np.asarray(nn_g, np.float32).reshape(P, 1),
        bx=np.asarray(sn_b, np.float32).reshape(P, 1),
        bn=np.asarray(nn_b, np.float32).reshape(P, 1),
    )

    in_maps = []
    for c in range(NCORES):
        m = dict(shared_map)
        m["xs"] = np.ascontiguousarray(percore["xs"][c])
        m["idx"] = np.ascontiguousarray(percore["idx"][c])
        m["dlb"] = np.ascontiguousarray(percore["dlb"][c])
        m["deg"] = np.ascontiguousarray(percore["deg"][c])
        in_maps.append(m)

    # the axon-tunneled device occasionally reports
    # NRT_EXEC_UNIT_UNRECOVERABLE on the first attempt; a retry recovers it
    import sys
    import time
    last_exc = None
    for _attempt in range(3):
        try:
            t0 = time.time()
            res = run_bass_kernel_spmd(nc, in_maps, core_ids=list(range(NCORES)))
            print(f"[kernel] attempt {_attempt} ok in {time.time()-t0:.1f}s",
                  file=sys.stderr)
            break
        except Exception as e:  # noqa: BLE001
            print(f"[kernel] attempt {_attempt} failed after "
                  f"{time.time()-t0:.1f}s: {str(e)[:200]}", file=sys.stderr)
            last_exc = e
    else:
        raise last_exc

    y = np.empty((Bb, N, P), dtype=np.float32)
    NTC = struct["NTC"]
    for c in range(NCORES):
        yc = np.asarray(res.results[c]["y"]).astype(np.float32)
        for k in range(NTC):
            t = tids[c][k]
            n0 = t * P
            n1 = min(n0 + P, N)
            if n1 <= n0:
                continue
            y[0, n0:n1, :] = yc[k * P : k * P + (n1 - n0), :P]
            y[1, n0:n1, :] = yc[k * P : k * P + (n1 - n0), P:]
    return y
